# revision 1
# baseline (speedup 1.0000x reference)
"""Trainium2 Bass kernel for nn_Event_Critic_Net (dual-branch GAT critic).

Math: the reference reads the GAT output only at the LAST node of each
graph (graphs are 32 contiguous nodes), so only edges whose dst is a
graph's last node contribute:

    out_g = sigmoid( (sum_n alpha[n] * x[n,:]) @ W + bias )
    alpha[n] = cnt[n]*exp(e[n]) / (sum_n cnt[n]*exp(e[n]) + 1e-16)
    e[n] = leaky_relu(x[n]. w_src + x[last(g)]. w_dst),  w_* = W @ att_*

cnt[n] = #edges (n -> last(g)).  KEY: nodes with cnt==0 contribute
nothing (numerator or denominator) and are dropped entirely on the
host.  Only ~7 of 32 nodes per graph survive, so graphs are packed
densely: whole graphs into 128-slot tiles (~14 graphs/tile, same
graph->slot assignment for both branches), T_SEL=40 tiles/core.
Shipped per branch: packed x (node-major + feat-major), 0/1 membership
masks (slot->graph, both orientations), packed x_last (feat-major),
slot counts.  ~4.7MB/core total vs 8.9MB unpacked.

Device pipeline per branch:
  1. advb: a_dst per graph-slot gd: 10 matmuls xlast2T chunk x w_dst.
  2. e_psum accumulation: a_src (20 xt-chunk matmuls, strided 2-col
     outputs) + a_dst scatter (40 maskT matmuls, rhs = advb slice)
     accumulate into e_ps [128, 40] — one PSUM region, no DVE adds.
  3. P = cnt * exp(leaky_relu(e))  (DVE stt + ACT exp + DVE mult)
  4. z = P (*) x  (stride-0 broadcast multiply, bf16)
  5. aggregation: per tile, mask_t [128,32] stationary x z-tile ->
     yps[32 rows, 66] with tile_position col groups; dense PSUM.
  6. normalize by ones-column, 10 PE transposes -> ynT [64, 1280],
     project (3 MMs), sigmoid.  Then sg_u*sg_d, mlp, DMA out.
Host un-permutes via the packing's graph->gd map (gd = ynT column).
"""

import numpy as np
from contextlib import ExitStack

NC = 8            # cores
N = 131072        # nodes total
G = 4096          # graphs
NPG = 32          # nodes per graph
S = 64            # state size
H = 128           # hidden size
NPC = N // NC     # 16384 nodes per core
GPC = G // NC     # 512 graphs per core
SA = 66           # x columns: 64 features | ones | zero pad
T_SEL = 40        # packed node tiles per core (>= needed ~31)
SLOTS = T_SEL * 128
NB = T_SEL // 4   # 10 gd blocks of 128
NCH = SLOTS // 256  # 20 xt chunks (2-block packed)
GD = NB * 128     # 1280 gd slots
NEG = 0.2

_CACHE = {}


def _build_module():
    import concourse.tile as tile
    from concourse import bacc, mybir
    from concourse.alu_op_type import AluOpType as Alu

    f32 = mybir.dt.float32
    bf16 = mybir.dt.bfloat16
    Act = mybir.ActivationFunctionType

    nc = bacc.Bacc("TRN2", target_bir_lowering=False, debug=False,
                   num_devices=NC)

    dram = {}

    def din(name, shape, dt=f32):
        dram[name] = nc.dram_tensor(name, shape, dt, kind="ExternalInput")

    for p in ("u", "d"):
        din(f"{p}_xab", [128, T_SEL * SA], bf16)
        din(f"{p}_xt", [128, SLOTS // 2], bf16)
        din(f"{p}_mk", [128, T_SEL * 32], bf16)
        din(f"{p}_mkT", [128, GD], bf16)
        din(f"{p}_xlT", [64, GD], bf16)
    din("cb", [128, 391], bf16)
    din("cf", [128, 92])
    out_dram = nc.dram_tensor("out", [1, GD], f32, kind="ExternalOutput")

    with tile.TileContext(nc) as tc, ExitStack() as ctx:
        const = ctx.enter_context(tc.tile_pool(name="const", bufs=1))
        xp = ctx.enter_context(tc.tile_pool(name="xp", bufs=2))
        wk = ctx.enter_context(tc.tile_pool(name="wk", bufs=2))
        ps1 = ctx.enter_context(tc.tile_pool(name="ps1", bufs=1, space="PSUM"))

        cb = const.tile([128, 391], bf16, tag="cb")
        nc.scalar.dma_start(cb[:], dram["cb"].ap())
        cf = const.tile([128, 92], f32, tag="cf")
        nc.sync.dma_start(cf[:], dram["cf"].ap())

        eps = cf[:, 4:5]
        mlpb = cf[0:1, 5:6]
        biases = {"u": cf[:, 6:7], "d": cf[:, 7:8]}
        cnts = {"u": cf[:, 8:48], "d": cf[:, 48:88]}
        Bm4 = cf[:, 88:92]
        wv2s = {"u": cb[:, 0:2], "d": cb[:, 2:4]}
        wd2s = {"u": cb[0:64, 4:5], "d": cb[0:64, 5:6]}
        Ws = {"u": cb[0:64, 6:134], "d": cb[0:64, 134:262]}
        mlpW = cb[:, 262:263]
        identb = cb[:, 263:391]

        st = {"u": {}, "d": {}}
        for p in ("u", "d"):
            s = st[p]
            s["xab"] = xp.tile([128, T_SEL * SA], bf16, tag="xab",
                               name=f"xab_{p}")
            s["xt"] = xp.tile([128, SLOTS // 2], bf16, tag="xt",
                              name=f"xt_{p}")
            s["mk"] = xp.tile([128, T_SEL * 32], bf16, tag="mk",
                              name=f"mk_{p}")
            s["mkT"] = xp.tile([128, GD], bf16, tag="mkT", name=f"mkT_{p}")
            s["xlT"] = xp.tile([64, GD], bf16, tag="xlT", name=f"xlT_{p}")

        # queue map: scalar + gpsimd carry bulk; sync only tiny cf
        for tname in ("xlT", "xt", "xab", "mkT", "mk"):
            nc.scalar.dma_start(st["u"][tname][:], dram[f"u_{tname}"].ap())
            nc.gpsimd.dma_start(st["d"][tname][:], dram[f"d_{tname}"].ap())

        # shared PSUM tiles (advb cols 0-19; a_dst scatter cols 20-99)
        epsud = ps1.tile([128, 2 * T_SEL], f32, tag="epsud")
        advb = ps1.tile([128, 2 * NB + 2 * T_SEL], f32, tag="advb")
        ypsA = ps1.tile([128, 7 * SA], f32, tag="ypsA")
        ypsB = ps1.tile([128, 3 * SA], f32, tag="ypsB")
        ytpA = ps1.tile([64, 7 * 128], bf16, tag="ytpA")
        ytpB = ps1.tile([64, 3 * 128], bf16, tag="ytpB")

        def branch_front(p, po):
            """advb, e_ps accumulation, P, z for branch p
            (po = 0 for u, 1 for d: offsets into shared PSUM tiles)."""
            s = st[p]
            # 1. a_dst per gd slot: 10 MMs
            for c in range(NB):
                nc.tensor.matmul(
                    advb[:, NB * po + c:NB * po + c + 1],
                    s["xlT"][:, 128 * c:128 * (c + 1)],
                    wd2s[p], start=True, stop=True)
            advs = wk.tile([128, NB], f32, tag="advs", name=f"advs_{p}")
            nc.scalar.activation(
                advs[:], advb[:, NB * po:NB * (po + 1)], Act.Copy)
            # block-masked advs columns: advm[:, 4c+n] =
            #   advs[:, c] if partition//32 == n else 0
            advm = wk.tile([128, 4 * NB], bf16, tag="advm",
                           name=f"advm_{p}")
            for c in range(NB):
                nc.vector.tensor_scalar(
                    advm[:, 4 * c:4 * c + 4], Bm4, advs[:, c:c + 1],
                    None, op0=Alu.mult)

            # 2a. a_src chunks (xt interleaves even/odd tiles, so
            # chunk c yields contiguous cols {2c, 2c+1})
            for c in range(NCH):
                nc.tensor.matmul(
                    epsud[:, T_SEL * po + 2 * c:T_SEL * po + 2 * c + 2],
                    s["xt"][:, 128 * c:128 * (c + 1)],
                    wv2s[p], start=True, stop=True)
            ep = epsud[:, T_SEL * po:T_SEL * (po + 1)]
            # 2b. a_dst scatter to slots (separate PSUM cols, no accum):
            # lhsT = mkT block [128, 128], rhs = masked advs cols [128, 4]
            adps = advb[:, 2 * NB + T_SEL * po:2 * NB + T_SEL * (po + 1)]
            for c in range(NB):
                nc.tensor.matmul(
                    adps[:, 4 * c:4 * c + 4],
                    s["mkT"][:, 128 * c:128 * (c + 1)],
                    advm[:, 4 * c:4 * c + 4],
                    start=True, stop=True)
            adsb = wk.tile([128, T_SEL], f32, tag="adsb")
            nc.scalar.activation(adsb[:], adps, Act.Copy)

            # 3. P = cnt * exp(leaky_relu(a_src + a_dst))
            esb = wk.tile([128, T_SEL], f32, tag="esb")
            nc.vector.tensor_tensor(esb[:], ep, adsb[:], op=Alu.add)
            e = wk.tile([128, T_SEL], f32, tag="e")
            nc.vector.scalar_tensor_tensor(
                e[:], esb[:], NEG, esb[:], op0=Alu.mult, op1=Alu.max)
            ex = wk.tile([128, T_SEL], f32, tag="ex")
            nc.scalar.activation(ex[:], e[:], Act.Exp)
            P = wk.tile([128, T_SEL], bf16, tag="P", name=f"P_{p}")
            nc.vector.tensor_tensor(P[:], ex[:], cnts[p], op=Alu.mult)

            # 4. z = P (*) xab (half on DVE, half on GPSIMD)
            z = wk.tile([128, T_SEL * SA], bf16, tag="z", name=f"z_{p}")
            s["z"] = z
            hT2 = T_SEL // 2
            for k, eng in ((0, nc.vector), (1, nc.vector)):
                eng.tensor_tensor(
                    z[:].rearrange("q (t s) -> q t s", s=SA)[
                        :, hT2 * k:hT2 * (k + 1), :],
                    s["xab"][:].rearrange("q (t s) -> q t s", s=SA)[
                        :, hT2 * k:hT2 * (k + 1), :],
                    P[:, hT2 * k:hT2 * (k + 1)].broadcast_to(
                        (128, hT2, SA)),
                    op=Alu.mult)

        def branch_agg(p):
            """aggregation + normalize + transpose + project + sigmoid."""
            s = st[p]
            for t in range(T_SEL):
                yb = ypsA if t < 28 else ypsB
                co = SA * (t // 4) if t < 28 else SA * ((t - 28) // 4)
                nc.tensor.matmul(
                    yb[32 * (t % 4):32 * (t % 4) + 32, co:co + SA],
                    s["mk"][:, 32 * t:32 * (t + 1)],
                    s["z"][:, SA * t:SA * (t + 1)],
                    start=True, stop=True,
                    tile_position=(0, 32 * (t % 4)))

            ysb = wk.tile([128, NB * SA], f32, tag="ysb")
            nc.scalar.activation(ysb[:, 0:7 * SA], ypsA[:], Act.Copy)
            nc.scalar.activation(ysb[:, 7 * SA:], ypsB[:], Act.Copy)
            dn = wk.tile([128, NB], f32, tag="dn")
            nc.vector.tensor_scalar(
                dn[:], ysb[:].rearrange("p (t s) -> p t s", s=SA)[:, :, S],
                eps, None, op0=Alu.add)
            rp = wk.tile([128, NB], f32, tag="rp")
            nc.vector.reciprocal_approx_fast(rp[:], dn[:])
            ynrm = wk.tile([128, NB * S], bf16, tag="ynrm")
            nc.vector.tensor_tensor(
                ynrm[:].rearrange("p (t s) -> p t s", s=S),
                ysb[:].rearrange("p (t s) -> p t s", s=SA)[:, :, 0:S],
                rp[:].broadcast_to((128, NB, S)),
                op=Alu.mult)
            for i in range(NB):
                yt = ytpA if i < 7 else ytpB
                co = 128 * i if i < 7 else 128 * (i - 7)
                nc.tensor.transpose(
                    yt[:, co:co + 128],
                    ynrm[:, S * i:S * (i + 1)],
                    identb, tile_position=(0, 0))
            ynT = wk.tile([64, GD], bf16, tag="ynT")
            nc.vector.tensor_copy(ynT[:, 0:7 * 128], ytpA[:])
            nc.vector.tensor_copy(ynT[:, 7 * 128:], ytpB[:])
            sg = wk.tile([H, GD], bf16, tag="sig", name=f"sig_{p}")
            hT = ps1.tile([H, 512], f32, tag="hT")
            for i in range(3):
                cw = 512 if i < 2 else 256
                nc.tensor.matmul(hT[:, 0:cw], Ws[p],
                                 ynT[:, 512 * i:512 * i + cw],
                                 start=True, stop=True)
                nc.scalar.activation(sg[:, 512 * i:512 * i + cw],
                                     hT[:, 0:cw], Act.Sigmoid,
                                     bias=biases[p])
            st[p]["sig"] = sg

        branch_front("u", 0)
        branch_front("d", 1)
        branch_agg("u")
        branch_agg("d")

        prod = wk.tile([H, GD], bf16, tag="prod")
        nc.vector.tensor_tensor(prod[:], st["u"]["sig"][:],
                                st["d"]["sig"][:], op=Alu.mult)
        o_sb = wk.tile([1, GD], f32, tag="o_sb")
        o_ps = ps1.tile([1, 512], f32, tag="mix")
        for i in range(3):
            cw = 512 if i < 2 else 256
            nc.tensor.matmul(o_ps[:, 0:cw], mlpW,
                             prod[:, 512 * i:512 * i + cw],
                             start=True, stop=True)
            nc.vector.tensor_scalar(
                o_sb[:, 512 * i:512 * i + cw], o_ps[:, 0:cw], mlpb, None,
                op0=Alu.add)
        nc.sync.dma_start(out_dram.ap(), o_sb[:])

    nc.compile()
    return nc


def _get_module():
    if "nc" not in _CACHE:
        _CACHE["nc"] = _build_module()
    return _CACHE["nc"]


def _nz_lists(ei):
    src = np.asarray(ei[0]).astype(np.int64)
    dst = np.asarray(ei[1]).astype(np.int64)
    valid = (dst % NPG) == (NPG - 1)
    cnt = np.bincount(src[valid], minlength=N).astype(np.float32)
    return cnt


def _pack_core(cnt_u, cnt_d, base):
    """Greedy whole-graph packing for one core.  Returns tiles: list of
    graph lists, and per-branch slot layouts."""
    ku = [(np.nonzero(cnt_u[base + NPG * g: base + NPG * (g + 1)])[0])
          for g in range(GPC)]
    kd = [(np.nonzero(cnt_d[base + NPG * g: base + NPG * (g + 1)])[0])
          for g in range(GPC)]
    tiles = []
    cur, su, sd = [], 0, 0
    for g in range(GPC):
        nu, nd = len(ku[g]), len(kd[g])
        if su + nu > 128 or sd + nd > 128 or len(cur) == 32:
            tiles.append(cur)
            cur, su, sd = [], 0, 0
        cur.append(g)
        su += nu
        sd += nd
    if cur:
        tiles.append(cur)
    assert len(tiles) <= T_SEL, f"packing needs {len(tiles)} tiles"
    return tiles, ku, kd


def _branch_arrays(tiles, klists, cnt, x, base):
    """Build packed arrays for one branch of one core."""
    import ml_dtypes
    bf = ml_dtypes.bfloat16
    xs = np.zeros((SLOTS, S), np.float32)
    cs = np.zeros(SLOTS, np.float32)
    mk = np.zeros((T_SEL, 128, 32), np.float32)
    mkT = np.zeros((128, GD), np.float32)
    for t, gs in enumerate(tiles):
        off = 0
        for j, g in enumerate(gs):
            nodes = base + NPG * g + klists[g]
            k = len(nodes)
            xs[128 * t + off:128 * t + off + k] = x[nodes]
            cs[128 * t + off:128 * t + off + k] = cnt[nodes]
            mk[t, off:off + k, j] = 1.0
            mkT[32 * (t % 4) + j, 128 * (t // 4) + off:
                128 * (t // 4) + off + k] = 1.0
            off += k

    xab = np.zeros((T_SEL, 128, SA), np.float32)
    xab[:, :, :S] = xs.reshape(T_SEL, 128, S)
    xab[:, :, S] = 1.0
    xab = np.ascontiguousarray(
        xab.transpose(1, 0, 2).reshape(128, T_SEL * SA)).astype(bf)
    # xt: block 0 = even tiles, block 1 = odd tiles (chunk c covers
    # tiles 2c and 2c+1 -> contiguous a_src output columns)
    xst = xs.reshape(T_SEL, 128, S)
    xtv = np.concatenate([
        xst[0::2].reshape(SLOTS // 2, S).T,
        xst[1::2].reshape(SLOTS // 2, S).T], axis=0)
    xtv = np.ascontiguousarray(xtv).astype(bf)
    cnt_t = np.ascontiguousarray(cs.reshape(T_SEL, 128).T)
    mk2 = np.ascontiguousarray(
        mk.transpose(1, 0, 2).reshape(128, T_SEL * 32)).astype(bf)
    return {"xab": xab, "xt": xtv, "mk": mk2, "mkT": mkT.astype(bf),
            "cnt": cnt_t}


def _build_in_maps(inputs):
    import ml_dtypes
    bf = ml_dtypes.bfloat16
    data = {}
    for p, pref in (("u", "up"), ("d", "down")):
        data[p] = {
            "x": np.asarray(inputs[f"{pref}_x"], np.float32),
            "cnt": _nz_lists(inputs[f"{pref}_edge_index"]),
            "W": np.asarray(inputs[f"{pref}_W"], np.float32),
            "ws": None, "wd": None,
            "bias": np.asarray(inputs[f"{pref}_bias"],
                               np.float32).reshape(H),
        }
        data[p]["ws"] = data[p]["W"] @ np.asarray(
            inputs[f"{pref}_att_src"], np.float32)
        data[p]["wd"] = data[p]["W"] @ np.asarray(
            inputs[f"{pref}_att_dst"], np.float32)

    cb = np.zeros((128, 391), np.float32)
    # wv2: col0/1 for u, col2/3 for d (2-block packed w_src)
    cb[:S, 0] = data["u"]["ws"]
    cb[S:, 1] = data["u"]["ws"]
    cb[:S, 2] = data["d"]["ws"]
    cb[S:, 3] = data["d"]["ws"]
    cb[0:S, 4] = data["u"]["wd"]
    cb[0:S, 5] = data["d"]["wd"]
    cb[0:S, 6:134] = data["u"]["W"]
    cb[0:S, 134:262] = data["d"]["W"]
    cb[:, 262] = np.asarray(inputs["mlp_W"], np.float32).reshape(H)
    cb[:, 263:391] = np.eye(128, dtype=np.float32)

    cf = np.zeros((128, 92), np.float32)
    cf[:, 4] = 1e-16
    cf[0, 5] = float(np.asarray(inputs["mlp_b"]).reshape(-1)[0])
    cf[:, 6] = data["u"]["bias"]
    cf[:, 7] = data["d"]["bias"]
    cf[np.arange(128), 88 + np.arange(128) // 32] = 1.0

    in_maps = []
    perms = []
    for c in range(NC):
        base = c * NPC
        tiles, ku, kd = _pack_core(data["u"]["cnt"], data["d"]["cnt"], base)
        m = {}
        gd_of_g = np.full(GPC, -1, np.int64)
        for t, gs in enumerate(tiles):
            for j, g in enumerate(gs):
                gd_of_g[g] = 32 * t + j
        assert (gd_of_g >= 0).all()
        perms.append(gd_of_g)
        for p, kl in (("u", ku), ("d", kd)):
            arrs = _branch_arrays(tiles, kl, data[p]["cnt"],
                                  data[p]["x"], base)
            for kk in ("xab", "xt", "mk", "mkT"):
                m[f"{p}_{kk}"] = arrs[kk]
            # xlT: feat-major packed x_last by gd slot
            xlT = np.zeros((S, GD), np.float32)
            gl = np.arange(GPC)
            last = data[p]["x"][base + NPG * gl + NPG - 1]   # [GPC, S]
            xlT[:, gd_of_g] = last.T
            m[f"{p}_xlT"] = xlT.astype(bf)
            if p == "u":
                cfc = cf.copy()
                cfc[:, 8:48] = arrs["cnt"]
                m["cf"] = cfc
            else:
                m["cf"][:, 48:88] = arrs["cnt"]
        m["cb"] = cb.astype(bf)
        in_maps.append(m)
    return in_maps, perms


def kernel(**inputs):
    from concourse.bass_utils import run_bass_kernel_spmd

    nc = _get_module()
    in_maps, perms = _build_in_maps(inputs)
    res = run_bass_kernel_spmd(nc, in_maps, core_ids=list(range(NC)))
    out = np.empty((NC, GPC), np.float32)
    for c, r in enumerate(res.results):
        full = np.asarray(r["out"], np.float32).reshape(GD)
        out[c] = full[perms[c]]
    return out.reshape(G, 1)



# revision 6
# speedup vs baseline: 1.1903x; 1.1903x over previous
"""Trainium2 Bass kernel for nn_Event_Critic_Net (dual-branch GAT critic).

Math: the reference reads the GAT output only at the LAST node of each
graph (graphs are 32 contiguous nodes), so only edges whose dst is a
graph's last node contribute:

    out_g = sigmoid( (sum_n alpha[n] * x[n,:]) @ W + bias )
    alpha[n] = cnt[n]*exp(e[n]) / (sum_n cnt[n]*exp(e[n]) + 1e-16)
    e[n] = leaky_relu(x[n]. w_src + x[last(g)]. w_dst),  w_* = W @ att_*

cnt[n] = #edges (n -> last(g)).  Nodes with cnt==0 are dropped on the
host; surviving nodes (~7/graph) are packed whole-graph into 128-slot
tiles via first-fit-decreasing (T=32 tiles/core, measured need <=30).

Device pipeline per branch (u then d, pipelined across engines):
  1. a_dst per graph-slot: DVE mult(x_last, wd) + reduce  -> advs.
  2. a_dst scatter to node slots: 8 mkT matmuls -> e_ps (PSUM).
  3. a_src: DVE mult(xab, ws) + reduce -> asrc; e = e_ps + asrc,
     leaky (DVE), exp (ACT), P = cnt*exp (DVE).
  4. z = P (*) xab (stride-0 broadcast multiply, bf16, reuses tmp).
  5. aggregation: per tile, mk [128,32] stationary x z-tile -> yps
     (tile_position row groups, col-block t//4; block 7 at col 512).
  6. normalize by ones-col (DVE from PSUM), 8 PE transposes -> ynT,
     project (2 MMs), tanh(h/2 + b/2) [same ACT table set as exp;
     sigmoid_u*sigmoid_d = 0.25(1+tanh_u)(1+tanh_d), 0.25 folded into
     mlpW host-side], prod (DVE), mlp (2 MMs), bias, DMA out.

DMA: 3 queues (sync/scalar HWDGE + gpsimd SWDGE), front tensors first;
8 dummy matmuls warm the PE HAM clock gate during the DMA wait.
Host un-permutes via the packing's graph->gd map (gd = out column).
"""

import numpy as np
from contextlib import ExitStack

NC = 8            # cores
N = 131072        # nodes total
G = 4096          # graphs
NPG = 32          # nodes per graph
S = 64            # state size
H = 128           # hidden size
NPC = N // NC     # 16384 nodes per core
GPC = G // NC     # 512 graphs per core
SA = 66           # x columns: 64 features | ones | zero pad
T = 32            # packed node tiles per core per branch (need <=30)
NB = T // 4       # 8 mkT blocks / gd blocks of 128
GD = NB * 128     # 1024 gd slots
NEG = 0.2

# cb (bf16) column layout
CB_ID = 0                      # identity [128,128]
CB_WSU = CB_ID + 128           # Ws_u rows 0:64 [*,128]
CB_WSD = CB_WSU + 128
CB_SRU = CB_WSD + 128          # wsrow_u [128,66] (ws bcast along partitions)
CB_SRD = CB_SRU + SA
CB_DRU = CB_SRD + SA           # wdrow_u [128,64]
CB_DRD = CB_DRU + S
CB_MLP = CB_DRD + S            # 0.25*mlpW [128,1]
CB_BM4 = CB_MLP + 1            # Bm4 [128,4]
CBW = CB_BM4 + 4

# cf (f32) column layout
CF_EPS = 0                     # 1e-16
CF_MLB = 1                     # mlp_b
CF_BU = 2                      # bias_u/2 per-partition
CF_BD = 3
CF_CNT = 4                     # cnt_u [128,T] then cnt_d [128,T]
CFW = CF_CNT + 2 * T

_CACHE = {}


def _build_module():
    import concourse.tile as tile
    from concourse import bacc, mybir
    from concourse.alu_op_type import AluOpType as Alu

    f32 = mybir.dt.float32
    bf16 = mybir.dt.bfloat16
    Act = mybir.ActivationFunctionType
    Ax = mybir.AxisListType

    nc = bacc.Bacc("TRN2", target_bir_lowering=False, debug=False,
                   num_devices=NC)

    dram = {}

    def din(name, shape, dt=bf16):
        dram[name] = nc.dram_tensor(name, shape, dt, kind="ExternalInput")

    for p in ("u", "d"):
        din(f"{p}_xab", [128, T * SA])
        din(f"{p}_mk", [128, T * 32])
        din(f"{p}_mkT", [128, NB * 128])
        din(f"{p}_xlb", [128, NB * S])
    din("cb", [128, CBW])
    din("cf", [128, CFW], f32)
    out_dram = nc.dram_tensor("out", [1, GD], f32, kind="ExternalOutput")

    with tile.TileContext(nc) as tc, ExitStack() as ctx:
        const = ctx.enter_context(tc.tile_pool(name="const", bufs=1))
        xp = ctx.enter_context(tc.tile_pool(name="xp", bufs=1))
        wk = ctx.enter_context(tc.tile_pool(name="wk", bufs=1))
        ps1 = ctx.enter_context(tc.tile_pool(name="ps1", bufs=1, space="PSUM"))

        cb = const.tile([128, CBW], bf16, tag="cb")
        cf = const.tile([128, CFW], f32, tag="cf")
        st = {"u": {}, "d": {}}
        for p in ("u", "d"):
            s = st[p]
            s["xab"] = xp.tile([128, T * SA], bf16, tag=f"xab_{p}", name=f"xab_{p}")
            s["mk"] = xp.tile([128, T * 32], bf16, tag=f"mk_{p}", name=f"mk_{p}")
            s["mkT"] = xp.tile([128, NB * 128], bf16, tag=f"mkT_{p}", name=f"mkT_{p}")
            s["xlb"] = xp.tile([128, NB * S], bf16, tag=f"xlb_{p}", name=f"xlb_{p}")

        # ---- DMA enqueues (3 queues; front tensors first) ----
        nc.sync.dma_start(cf[:], dram["cf"].ap())
        nc.sync.dma_start(cb[:], dram["cb"].ap())
        nc.sync.dma_start(st["u"]["xlb"][:], dram["u_xlb"].ap())
        nc.sync.dma_start(st["u"]["mkT"][:], dram["u_mkT"].ap())
        nc.sync.dma_start(st["d"]["xlb"][:], dram["d_xlb"].ap())
        nc.sync.dma_start(st["d"]["mkT"][:], dram["d_mkT"].ap())
        nc.scalar.dma_start(st["u"]["xab"][:], dram["u_xab"].ap())
        nc.scalar.dma_start(st["d"]["xab"][:], dram["d_xab"].ap())
        nc.gpsimd.dma_start(st["u"]["mk"][:], dram["u_mk"].ap())
        nc.gpsimd.dma_start(st["d"]["mk"][:], dram["d_mk"].ap())

        identb = cb[:, CB_ID:CB_ID + 128]
        Ws = {"u": cb[0:S, CB_WSU:CB_WSU + 128],
              "d": cb[0:S, CB_WSD:CB_WSD + 128]}
        wsrow = {"u": cb[:, CB_SRU:CB_SRU + SA],
                 "d": cb[:, CB_SRD:CB_SRD + SA]}
        wdrow = {"u": cb[:, CB_DRU:CB_DRU + S],
                 "d": cb[:, CB_DRD:CB_DRD + S]}
        mlpW = cb[:, CB_MLP:CB_MLP + 1]
        Bm4 = cb[:, CB_BM4:CB_BM4 + 4]
        eps = cf[:, CF_EPS:CF_EPS + 1]
        mlpb = cf[0:1, CF_MLB:CF_MLB + 1]
        biases = {"u": cf[:, CF_BU:CF_BU + 1], "d": cf[:, CF_BD:CF_BD + 1]}
        cnts = {"u": cf[:, CF_CNT:CF_CNT + T],
                "d": cf[:, CF_CNT + T:CF_CNT + 2 * T]}

        # ---- PSUM tiles (8 banks exactly) ----
        epsud = ps1.tile([128, 2 * T], f32, tag="epsud")
        # per-branch y PSUM: blocks 0..6 at col 66*b, block 7 at col 512
        yps = {"u": ps1.tile([128, 578], f32, tag="yps_u", name="yps_u"),
               "d": ps1.tile([128, 578], f32, tag="yps_d", name="yps_d")}
        ytp = ps1.tile([64, NB * 128], bf16, tag="ytp")      # shared u->d
        hT = ps1.tile([128, 1024], f32, tag="hT")            # dummies/proj/mlp

        def ycol(b):
            return SA * b if b < 7 else 512

        # ---- SBUF work tiles ----
        zt = {p: wk.tile([128, T * SA], bf16, tag=f"zt_{p}", name=f"zt_{p}") for p in "ud"}
        advt = {p: wk.tile([128, NB * S], bf16, tag=f"advt_{p}", name=f"advt_{p}") for p in "ud"}
        advs = {p: wk.tile([128, NB], f32, tag=f"advs_{p}", name=f"advs_{p}") for p in "ud"}
        advm = {p: wk.tile([128, NB * 4], bf16, tag=f"advm_{p}", name=f"advm_{p}") for p in "ud"}
        asrc = {p: wk.tile([128, T], f32, tag=f"asrc_{p}", name=f"asrc_{p}") for p in "ud"}
        ee = {p: wk.tile([128, T], f32, tag=f"ee_{p}", name=f"ee_{p}") for p in "ud"}
        ex = {p: wk.tile([128, T], f32, tag=f"ex_{p}", name=f"ex_{p}") for p in "ud"}
        Pt = {p: wk.tile([128, T], f32, tag=f"P_{p}", name=f"P_{p}") for p in "ud"}
        dn = {p: wk.tile([128, NB], f32, tag=f"dn_{p}", name=f"dn_{p}") for p in "ud"}
        rp = {p: wk.tile([128, NB], f32, tag=f"rp_{p}", name=f"rp_{p}") for p in "ud"}
        ynrm = {p: wk.tile([128, NB * S], bf16, tag=f"ynrm_{p}", name=f"ynrm_{p}") for p in "ud"}
        ynT = {p: wk.tile([64, GD], bf16, tag=f"ynT_{p}", name=f"ynT_{p}") for p in "ud"}
        sg = {p: wk.tile([128, GD], bf16, tag=f"sg_{p}", name=f"sg_{p}") for p in "ud"}
        prod = wk.tile([128, GD], bf16, tag="prod")
        o_sb = wk.tile([1, GD], f32, tag="o_sb")

        # ---- PE HAM warmup: 8 dummy matmuls while DMA lands ----
        for i in range(8):
            nc.tensor.matmul(hT[:, 0:512], identb,
                             cb[:, 0:512], start=True, stop=True)

        def front(p, po):
            s = st[p]
            # a_dst = sum_f x_last * wd   (DVE mult + reduce)
            nc.vector.tensor_tensor(
                advt[p][:].rearrange("q (b f) -> q b f", f=S),
                s["xlb"][:].rearrange("q (b f) -> q b f", f=S),
                wdrow[p][:, None, :].broadcast_to((128, NB, S)),
                op=Alu.mult)
            nc.vector.tensor_reduce(
                advs[p][:], advt[p][:].rearrange("q (b f) -> q b f", f=S),
                axis=Ax.X, op=Alu.add)
            # advm[:, 4b+n] = advs[:, b] masked to partition group n
            nc.vector.tensor_tensor(
                advm[p][:].rearrange("q (b n) -> q b n", n=4),
                advs[p][:, :, None].broadcast_to((128, NB, 4)),
                Bm4[:, None, :].broadcast_to((128, NB, 4)),
                op=Alu.mult)
            # scatter a_dst to slots: PSUM cols [po*T + 4b : +4]
            for b in range(NB):
                nc.tensor.matmul(
                    epsud[:, T * po + 4 * b:T * po + 4 * b + 4],
                    s["mkT"][:, 128 * b:128 * (b + 1)],
                    advm[p][:, 4 * b:4 * b + 4],
                    start=True, stop=True)
            # a_src = sum_f xab * ws   (mult into zt tmp, then reduce)
            nc.vector.tensor_tensor(
                zt[p][:].rearrange("q (t f) -> q t f", f=SA),
                s["xab"][:].rearrange("q (t f) -> q t f", f=SA),
                wsrow[p][:, None, :].broadcast_to((128, T, SA)),
                op=Alu.mult)
            nc.vector.tensor_reduce(
                asrc[p][:], zt[p][:].rearrange("q (t f) -> q t f", f=SA),
                axis=Ax.X, op=Alu.add)
            # e = leaky(asrc + a_dst); P = cnt * exp(e)
            nc.vector.scalar_tensor_tensor(
                ee[p][:], epsud[:, T * po:T * (po + 1)], 1.0, asrc[p][:],
                op0=Alu.mult, op1=Alu.add)
            nc.vector.scalar_tensor_tensor(
                ee[p][:], ee[p][:], NEG, ee[p][:], op0=Alu.mult, op1=Alu.max)
            nc.scalar.activation(ex[p][:], ee[p][:], Act.Exp)
            nc.vector.tensor_tensor(Pt[p][:], ex[p][:], cnts[p], op=Alu.mult)
            # z = P (*) xab  (overwrites zt; ones-col becomes P)
            nc.vector.tensor_tensor(
                zt[p][:].rearrange("q (t f) -> q t f", f=SA),
                s["xab"][:].rearrange("q (t f) -> q t f", f=SA),
                Pt[p][:, :, None].broadcast_to((128, T, SA)),
                op=Alu.mult)

        def agg(p):
            s = st[p]
            yb = yps[p]
            for t in range(T):
                nc.tensor.matmul(
                    yb[32 * (t % 4):32 * (t % 4) + 32,
                       ycol(t // 4):ycol(t // 4) + SA],
                    s["mk"][:, 32 * t:32 * (t + 1)],
                    zt[p][:, SA * t:SA * (t + 1)],
                    start=True, stop=True,
                    tile_position=(0, 32 * (t % 4)))

        def norm_proj(p, po):
            yb = yps[p]
            # denominators (ones col = S) + eps; blocks 0-6 strided, 7 apart
            nc.vector.tensor_scalar(
                dn[p][:, 0:7],
                yb[:, 0:7 * SA].rearrange("q (b f) -> q b f", f=SA)[:, :, S],
                eps, None, op0=Alu.add)
            nc.vector.tensor_scalar(
                dn[p][:, 7:8], yb[:, 512 + S:512 + S + 1], eps, None,
                op0=Alu.add)
            nc.vector.reciprocal_approx_fast(rp[p][:], dn[p][:])
            nc.vector.tensor_tensor(
                ynrm[p][:].rearrange("q (b f) -> q b f", f=S)[:, 0:7],
                yb[:, 0:7 * SA].rearrange("q (b f) -> q b f", f=SA)[:, :, 0:S],
                rp[p][:, 0:7, None].broadcast_to((128, 7, S)),
                op=Alu.mult)
            nc.vector.tensor_tensor(
                ynrm[p][:, 7 * S:8 * S],
                yb[:, 512:512 + S],
                rp[p][:, 7:8].broadcast_to((128, S)),
                op=Alu.mult)
            for b in range(NB):
                nc.tensor.transpose(
                    ytp[:, 128 * b:128 * (b + 1)],
                    ynrm[p][:, S * b:S * (b + 1)],
                    identb, tile_position=(0, 0))
            nc.vector.tensor_copy(ynT[p][:], ytp[:])
            for i in range(2):
                nc.tensor.matmul(hT[:, 512 * i:512 * (i + 1)], Ws[p],
                                 ynT[p][:, 512 * i:512 * (i + 1)],
                                 start=True, stop=True)
            # sigmoid(h+b) = 0.5*(1+tanh(h/2+b/2)); 0.25 folded into mlpW
            nc.scalar.activation(sg[p][:], hT[:], Act.Tanh,
                                 bias=biases[p], scale=0.5)

        front("u", 0)
        front("d", 1)
        agg("u")
        norm_proj("u", 0)
        agg("d")
        norm_proj("d", 1)

        # (1+tanh_u)*(1+tanh_d): t1 = tu+1; prod = (td+1)*t1
        t1 = wk.tile([128, GD], bf16, tag="t1")
        nc.vector.scalar_tensor_tensor(
            t1[:], sg["u"][:], 1.0, sg["u"][:], op0=Alu.add, op1=Alu.bypass)
        nc.vector.scalar_tensor_tensor(
            prod[:], sg["d"][:], 1.0, t1[:], op0=Alu.add, op1=Alu.mult)
        for i in range(2):
            nc.tensor.matmul(hT[0:1, 512 * i:512 * (i + 1)], mlpW,
                             prod[:, 512 * i:512 * (i + 1)],
                             start=True, stop=True)
        nc.vector.tensor_scalar(
            o_sb[:], hT[0:1, :], mlpb, None, op0=Alu.add)
        nc.sync.dma_start(out_dram.ap(), o_sb[:])

    nc.compile()
    return nc


def _get_module():
    if "nc" not in _CACHE:
        _CACHE["nc"] = _build_module()
    return _CACHE["nc"]


def _nz_counts(ei):
    src = np.asarray(ei[0]).astype(np.int64)
    dst = np.asarray(ei[1]).astype(np.int64)
    valid = (dst % NPG) == (NPG - 1)
    return np.bincount(src[valid], minlength=N).astype(np.float32)


def _pack_core(cnt_u, cnt_d, base):
    """First-fit-decreasing whole-graph packing for one core (shared
    graph->tile map for both branches).  Returns tiles (list of graph
    lists) and per-graph node lists."""
    ku = [np.nonzero(cnt_u[base + NPG * g: base + NPG * (g + 1)])[0]
          for g in range(GPC)]
    kd = [np.nonzero(cnt_d[base + NPG * g: base + NPG * (g + 1)])[0]
          for g in range(GPC)]
    nu = np.array([len(k) for k in ku])
    nd = np.array([len(k) for k in kd])
    order = np.argsort(-(nu + nd), kind="stable")
    bins = []            # [su, sd, [graphs]]
    for g in order:
        placed = False
        for b in bins:
            if b[0] + nu[g] <= 128 and b[1] + nd[g] <= 128 and len(b[2]) < 32:
                b[0] += nu[g]
                b[1] += nd[g]
                b[2].append(g)
                placed = True
                break
        if not placed:
            bins.append([nu[g], nd[g], [g]])
    assert len(bins) <= T, f"packing needs {len(bins)} tiles > {T}"
    return [b[2] for b in bins], ku, kd


def _branch_arrays(tiles, klists, cnt, x, base):
    """Packed arrays for one branch of one core."""
    import ml_dtypes
    bf = ml_dtypes.bfloat16
    SLOTS = T * 128
    xs = np.zeros((SLOTS, S), np.float32)
    cs = np.zeros(SLOTS, np.float32)
    mk = np.zeros((T, 128, 32), np.float32)
    mkT = np.zeros((128, NB * 128), np.float32)
    for t, gs in enumerate(tiles):
        off = 0
        for j, g in enumerate(gs):
            nodes = base + NPG * g + klists[g]
            k = len(nodes)
            xs[128 * t + off:128 * t + off + k] = x[nodes]
            cs[128 * t + off:128 * t + off + k] = cnt[nodes]
            mk[t, off:off + k, j] = 1.0
            mkT[32 * (t % 4) + j, 128 * (t // 4) + off:
                128 * (t // 4) + off + k] = 1.0
            off += k

    xab = np.zeros((T, 128, SA), np.float32)
    xab[:, :, :S] = xs.reshape(T, 128, S)
    xab[:, :, S] = 1.0
    xab = np.ascontiguousarray(
        xab.transpose(1, 0, 2).reshape(128, T * SA)).astype(bf)
    cnt_t = np.ascontiguousarray(cs.reshape(T, 128).T)
    mk2 = np.ascontiguousarray(
        mk.transpose(1, 0, 2).reshape(128, T * 32)).astype(bf)
    return {"xab": xab, "mk": mk2, "mkT": mkT.astype(bf), "cnt": cnt_t}


def _build_in_maps(inputs):
    import ml_dtypes
    bf = ml_dtypes.bfloat16
    data = {}
    for p, pref in (("u", "up"), ("d", "down")):
        W = np.asarray(inputs[f"{pref}_W"], np.float32)
        data[p] = {
            "x": np.asarray(inputs[f"{pref}_x"], np.float32),
            "cnt": _nz_counts(inputs[f"{pref}_edge_index"]),
            "ws": W @ np.asarray(inputs[f"{pref}_att_src"], np.float32),
            "wd": W @ np.asarray(inputs[f"{pref}_att_dst"], np.float32),
            "W": W,
            "bias": np.asarray(inputs[f"{pref}_bias"], np.float32).reshape(H),
        }

    cb = np.zeros((128, CBW), np.float32)
    cb[:, CB_ID:CB_ID + 128] = np.eye(128)
    cb[0:S, CB_WSU:CB_WSU + 128] = data["u"]["W"]
    cb[0:S, CB_WSD:CB_WSD + 128] = data["d"]["W"]
    cb[:, CB_SRU:CB_SRU + S] = data["u"]["ws"][None, :]
    cb[:, CB_SRD:CB_SRD + S] = data["d"]["ws"][None, :]
    cb[:, CB_DRU:CB_DRU + S] = data["u"]["wd"][None, :]
    cb[:, CB_DRD:CB_DRD + S] = data["d"]["wd"][None, :]
    cb[:, CB_MLP] = 0.25 * np.asarray(inputs["mlp_W"], np.float32).reshape(H)
    cb[np.arange(128), CB_BM4 + np.arange(128) // 32] = 1.0
    cb = cb.astype(bf)

    cf = np.zeros((128, CFW), np.float32)
    cf[:, CF_EPS] = 1e-16
    cf[0, CF_MLB] = float(np.asarray(inputs["mlp_b"]).reshape(-1)[0])
    cf[:, CF_BU] = 0.5 * data["u"]["bias"]
    cf[:, CF_BD] = 0.5 * data["d"]["bias"]

    in_maps = []
    perms = []
    for c in range(NC):
        base = c * NPC
        tiles, ku, kd = _pack_core(data["u"]["cnt"], data["d"]["cnt"], base)
        gd_of_g = np.full(GPC, -1, np.int64)
        for t, gs in enumerate(tiles):
            for j, g in enumerate(gs):
                gd_of_g[g] = 32 * t + j
        assert (gd_of_g >= 0).all()
        perms.append(gd_of_g)
        m = {"cb": cb}
        cfc = cf.copy()
        for p, kl in (("u", ku), ("d", kd)):
            arrs = _branch_arrays(tiles, kl, data[p]["cnt"],
                                  data[p]["x"], base)
            m[f"{p}_xab"] = arrs["xab"]
            m[f"{p}_mk"] = arrs["mk"]
            m[f"{p}_mkT"] = arrs["mkT"]
            off = CF_CNT if p == "u" else CF_CNT + T
            cfc[:, off:off + T] = arrs["cnt"]
            # x_last in gd-partition layout: xlb[p, b*S+f] = x_last[128b+p]
            xlb = np.zeros((128, NB * S), np.float32)
            gl = np.arange(GPC)
            last = data[p]["x"][base + NPG * gl + NPG - 1]   # [GPC, S]
            gd = gd_of_g
            xlb[(gd % 128)[:, None],
                (gd // 128)[:, None] * S + np.arange(S)[None, :]] = last
            m[f"{p}_xlb"] = xlb.astype(bf)
        m["cf"] = cfc
        in_maps.append(m)
    return in_maps, perms


def kernel(**inputs):
    from concourse.bass_utils import run_bass_kernel_spmd

    nc = _get_module()
    in_maps, perms = _build_in_maps(inputs)
    res = run_bass_kernel_spmd(nc, in_maps, core_ids=list(range(NC)))
    out = np.empty((NC, GPC), np.float32)
    for c, r in enumerate(res.results):
        full = np.asarray(r["out"], np.float32).reshape(GD)
        out[c] = full[perms[c]]
    return out.reshape(G, 1)


# revision 10
# speedup vs baseline: 1.2009x; 1.0089x over previous
"""Trainium2 Bass kernel for nn_Event_Critic_Net (dual-branch GAT critic).

Math: the reference reads the GAT output only at the LAST node of each
graph (graphs are 32 contiguous nodes), so only edges whose dst is a
graph's last node contribute:

    out_g = sigmoid( (sum_n alpha[n] * x[n,:]) @ W + bias )
    alpha[n] = cnt[n]*exp(e[n]) / (sum_n cnt[n]*exp(e[n]) + 1e-16)
    e[n] = leaky_relu(x[n]. w_src + x[last(g)]. w_dst),  w_* = W @ att_*

cnt[n] = #edges (n -> last(g)).  Nodes with cnt==0 are dropped on the
host; survivors (~7/graph) are packed whole-graph into 128-slot tiles
via first-fit-decreasing (T=32 tiles/core/branch, measured need <=30).

Device pipeline per branch (u then d, pipelined across engines):
  1. advb MMs: a_dst per gd slot (xlT chunks x wd2) -> PSUM.
  2. a_src MMs: 16 two-block xt chunks x wv2 -> e PSUM (start, no stop)
  3. advs copy (ACT), advm = advs*Bm4 (DVE), 8 mkT scatter MMs
     accumulate a_dst INTO the same e PSUM cols (stop=True).
  4. leaky (DVE from PSUM), exp (ACT), P = cnt*exp (DVE).
  5. mkP = mk (*) P : mask columns scaled by P (DVE); aggregation MMs
     use mkP stationary x raw xab tiles -> yps (ones col -> denom).
  6. normalize (DVE from PSUM), 8 PE transposes -> ynT, project (2 MMs),
     sigmoid (ACT; table-set load prefetched right after the last exp
     via a dummy 1-elem sigmoid), prod (DVE), mlp (2 MMs), bias, DMA.

DMA: 3 queues (sync/scalar HWDGE, gpsimd SWDGE); masks shipped as
fp8_e4m3 and cast to bf16 during the gpsimd DMA (halves their bytes).
Host un-permutes via the packing's graph->gd map (gd = out column).
"""

import numpy as np
from contextlib import ExitStack

NC = 8            # cores
N = 131072        # nodes total
G = 4096          # graphs
NPG = 32          # nodes per graph
S = 64            # state size
H = 128           # hidden size
NPC = N // NC     # 16384 nodes per core
GPC = G // NC     # 512 graphs per core
SA = 66           # x columns: 64 features | ones | zero pad
T = 32            # packed node tiles per core per branch (need <=30)
NB = T // 4       # 8 mkT blocks / gd blocks of 128
NCH = T // 2      # 16 two-block xt chunks
GD = NB * 128     # 1024 gd slots
NEG = 0.2

# cb (bf16) column layout
CB_ID = 0                      # identity [128,128]
CB_WSU = CB_ID + 128           # Ws_u rows 0:64 [*,128]
CB_WSD = CB_WSU + 128
CB_V2U = CB_WSD + 128          # wv2_u [128,2] (2-block ws)
CB_V2D = CB_V2U + 2
CB_D2U = CB_V2D + 2            # wd2_u [64,1]
CB_D2D = CB_D2U + 1
CB_MLP = CB_D2D + 1            # mlpW [128,1]
CB_BM4 = CB_MLP + 1            # Bm4 [128,4]
CBW = CB_BM4 + 4

# cf (f32) column layout
CF_EPS = 0                     # 1e-16
CF_MLB = 1                     # mlp_b
CF_BU = 2                      # bias_u per-partition
CF_BD = 3
CF_CNT = 4                     # cnt_u [128,T] then cnt_d [128,T]
CFW = CF_CNT + 2 * T

_CACHE = {}


def _build_module():
    import concourse.tile as tile
    from concourse import bacc, mybir
    from concourse.alu_op_type import AluOpType as Alu

    f32 = mybir.dt.float32
    bf16 = mybir.dt.bfloat16
    f8 = mybir.dt.float8e4
    Act = mybir.ActivationFunctionType

    nc = bacc.Bacc("TRN2", target_bir_lowering=False, debug=False,
                   num_devices=NC)

    dram = {}

    def din(name, shape, dt=bf16):
        dram[name] = nc.dram_tensor(name, shape, dt, kind="ExternalInput")

    for p in ("u", "d"):
        din(f"{p}_xab", [128, T * SA])
        din(f"{p}_xt", [128, NCH * 128])
        din(f"{p}_xlT", [64, GD])
        din(f"{p}_mk", [128, T * 32])
        din(f"{p}_mkT", [128, NB * 128])
    din("cb", [128, CBW])
    din("cf", [128, CFW], f32)
    out_dram = nc.dram_tensor("out", [1, GD], f32, kind="ExternalOutput")

    with tile.TileContext(nc) as tc, ExitStack() as ctx:
        const = ctx.enter_context(tc.tile_pool(name="const", bufs=1))
        xp = ctx.enter_context(tc.tile_pool(name="xp", bufs=1))
        wk = ctx.enter_context(tc.tile_pool(name="wk", bufs=1))
        ps1 = ctx.enter_context(tc.tile_pool(name="ps1", bufs=1, space="PSUM"))

        cb = const.tile([128, CBW], bf16, tag="cb")
        cf = const.tile([128, CFW], f32, tag="cf")
        st = {"u": {}, "d": {}}
        for p in ("u", "d"):
            s = st[p]
            s["xab"] = xp.tile([128, T * SA], bf16, tag=f"xab_{p}",
                               name=f"xab_{p}")
            s["xt"] = xp.tile([128, NCH * 128], bf16, tag=f"xt_{p}",
                              name=f"xt_{p}")
            s["xlT"] = xp.tile([64, GD], bf16, tag=f"xlT_{p}",
                               name=f"xlT_{p}")
            s["mk"] = xp.tile([128, T * 32], bf16, tag=f"mk_{p}",
                              name=f"mk_{p}")
            s["mkT"] = xp.tile([128, NB * 128], bf16, tag=f"mkT_{p}",
                               name=f"mkT_{p}")

        # ---- DMA enqueues (3 queues; front tensors first) ----
        nc.sync.dma_start(cf[:], dram["cf"].ap())
        nc.sync.dma_start(cb[:], dram["cb"].ap())
        nc.sync.dma_start(st["u"]["xlT"][:], dram["u_xlT"].ap())
        nc.sync.dma_start(st["u"]["xt"][:], dram["u_xt"].ap())
        nc.sync.dma_start(st["d"]["xlT"][:], dram["d_xlT"].ap())
        nc.scalar.dma_start(st["u"]["xab"][:], dram["u_xab"].ap())
        nc.scalar.dma_start(st["d"]["xt"][:], dram["d_xt"].ap())
        nc.gpsimd.dma_start(st["u"]["mkT"][:], dram["u_mkT"].ap())
        nc.gpsimd.dma_start(st["u"]["mk"][:], dram["u_mk"].ap())
        nc.gpsimd.dma_start(st["d"]["mkT"][:], dram["d_mkT"].ap())
        nc.gpsimd.dma_start(st["d"]["mk"][:], dram["d_mk"].ap())
        nc.gpsimd.dma_start(st["d"]["xab"][:], dram["d_xab"].ap())

        identb = cb[:, CB_ID:CB_ID + 128]
        Ws = {"u": cb[0:S, CB_WSU:CB_WSU + 128],
              "d": cb[0:S, CB_WSD:CB_WSD + 128]}
        wv2 = {"u": cb[:, CB_V2U:CB_V2U + 2], "d": cb[:, CB_V2D:CB_V2D + 2]}
        wd2 = {"u": cb[0:S, CB_D2U:CB_D2U + 1],
               "d": cb[0:S, CB_D2D:CB_D2D + 1]}
        mlpW = cb[:, CB_MLP:CB_MLP + 1]
        Bm4 = cb[:, CB_BM4:CB_BM4 + 4]
        eps = cf[:, CF_EPS:CF_EPS + 1]
        mlpb = cf[0:1, CF_MLB:CF_MLB + 1]
        biases = {"u": cf[:, CF_BU:CF_BU + 1], "d": cf[:, CF_BD:CF_BD + 1]}
        cnts = {"u": cf[:, CF_CNT:CF_CNT + T],
                "d": cf[:, CF_CNT + T:CF_CNT + 2 * T]}

        # ---- PSUM tiles (8 banks exactly) ----
        # epsud: e_u | e_d | advb_u | advb_d
        epsud = ps1.tile([128, 4 * T + 2 * NB], f32, tag="epsud")
        # per-branch y PSUM: blocks 0..6 at col 66*b, block 7 at col 512
        yps = {"u": ps1.tile([128, 578], f32, tag="yps_u", name="yps_u"),
               "d": ps1.tile([128, 578], f32, tag="yps_d", name="yps_d")}
        ytp = ps1.tile([64, NB * 128], bf16, tag="ytp")      # shared u->d
        hT = ps1.tile([128, 1024], f32, tag="hT")            # proj + mlp

        def ycol(b):
            return SA * b if b < 7 else 512

        # ---- SBUF work tiles ----
        advs = {p: wk.tile([128, NB], f32, tag=f"advs_{p}", name=f"advs_{p}")
                for p in "ud"}
        advm = {p: wk.tile([128, NB * 4], bf16, tag=f"advm_{p}",
                           name=f"advm_{p}") for p in "ud"}
        ee = {p: wk.tile([128, T], f32, tag=f"ee_{p}", name=f"ee_{p}")
              for p in "ud"}
        es = {p: wk.tile([128, T], f32, tag=f"es_{p}", name=f"es_{p}")
              for p in "ud"}
        adsb = {p: wk.tile([128, T], f32, tag=f"adsb_{p}", name=f"adsb_{p}")
                for p in "ud"}
        ex = {p: wk.tile([128, T], f32, tag=f"ex_{p}", name=f"ex_{p}")
              for p in "ud"}
        Pt = {p: wk.tile([128, T], f32, tag=f"P_{p}", name=f"P_{p}")
              for p in "ud"}
        mkP = {p: wk.tile([128, T * 32], bf16, tag=f"mkP_{p}",
                          name=f"mkP_{p}") for p in "ud"}
        dn = {p: wk.tile([128, NB], f32, tag=f"dn_{p}", name=f"dn_{p}")
              for p in "ud"}
        rp = {p: wk.tile([128, NB], f32, tag=f"rp_{p}", name=f"rp_{p}")
              for p in "ud"}
        ynrm = {p: wk.tile([128, NB * S], bf16, tag=f"ynrm_{p}",
                           name=f"ynrm_{p}") for p in "ud"}
        ynT = {p: wk.tile([64, GD], bf16, tag=f"ynT_{p}", name=f"ynT_{p}")
               for p in "ud"}
        sg = {p: wk.tile([128, GD], bf16, tag=f"sg_{p}", name=f"sg_{p}")
              for p in "ud"}
        prod = wk.tile([128, GD], bf16, tag="prod")
        o_sb = wk.tile([1, GD], f32, tag="o_sb")
        dum = wk.tile([1, 1], f32, tag="dum")

        def front(p, po):
            s = st[p]
            # a_dst per gd: 8 xlT-chunk MMs -> advb cols
            for b in range(NB):
                nc.tensor.matmul(
                    epsud[:, 4 * T + NB * po + b:4 * T + NB * po + b + 1],
                    s["xlT"][:, 128 * b:128 * (b + 1)],
                    wd2[p], start=True, stop=True)
            # a_src: 16 chunk MMs open the e accumulation group
            for c in range(NCH):
                nc.tensor.matmul(
                    epsud[:, T * po + 2 * c:T * po + 2 * c + 2],
                    s["xt"][:, 128 * c:128 * (c + 1)],
                    wv2[p], start=True, stop=True)
            # advs (ACT copy from PSUM), advm = advs * Bm4 (DVE)
            nc.scalar.activation(
                advs[p][:], epsud[:, 4 * T + NB * po:4 * T + NB * (po + 1)],
                Act.Copy)
            nc.vector.tensor_tensor(
                advm[p][:].rearrange("q (b n) -> q b n", n=4),
                advs[p][:, :, None].broadcast_to((128, NB, 4)),
                Bm4[:, None, :].broadcast_to((128, NB, 4)),
                op=Alu.mult)
            # scatter a_dst into the e cols (accumulate, close group)
            for b in range(NB):
                nc.tensor.matmul(
                    epsud[:, 2 * T + T * po + 4 * b:2 * T + T * po + 4 * b + 4],
                    s["mkT"][:, 128 * b:128 * (b + 1)],
                    advm[p][:, 4 * b:4 * b + 4],
                    start=True, stop=True)
            # e = e_src + e_dst; P = cnt * exp(leaky(e))
            nc.scalar.activation(
                adsb[p][:], epsud[:, 2 * T + T * po:2 * T + T * (po + 1)],
                Act.Copy)
            nc.vector.scalar_tensor_tensor(
                es[p][:], epsud[:, T * po:T * (po + 1)], 1.0, adsb[p][:],
                op0=Alu.mult, op1=Alu.add)
            nc.vector.scalar_tensor_tensor(
                ee[p][:], es[p][:], NEG, es[p][:], op0=Alu.mult, op1=Alu.max)
            nc.scalar.activation(ex[p][:], ee[p][:], Act.Exp)
            nc.vector.tensor_tensor(Pt[p][:], ex[p][:], cnts[p], op=Alu.mult)
            # mkP = mk (*) P  (mask columns scaled by P)
            nc.vector.tensor_tensor(
                mkP[p][:].rearrange("q (t j) -> q t j", j=32),
                s["mk"][:].rearrange("q (t j) -> q t j", j=32),
                Pt[p][:, :, None].broadcast_to((128, T, 32)),
                op=Alu.mult)

        def agg(p):
            s = st[p]
            yb = yps[p]
            for t in range(T):
                nc.tensor.matmul(
                    yb[32 * (t % 4):32 * (t % 4) + 32,
                       ycol(t // 4):ycol(t // 4) + SA],
                    mkP[p][:, 32 * t:32 * (t + 1)],
                    s["xab"][:, SA * t:SA * (t + 1)],
                    start=True, stop=True,
                    tile_position=(0, 32 * (t % 4)))

        def norm_proj(p):
            yb = yps[p]
            nc.vector.tensor_scalar(
                dn[p][:, 0:7],
                yb[:, 0:7 * SA].rearrange("q (b f) -> q b f", f=SA)[:, :, S],
                eps, None, op0=Alu.add)
            nc.vector.tensor_scalar(
                dn[p][:, 7:8], yb[:, 512 + S:512 + S + 1], eps, None,
                op0=Alu.add)
            nc.vector.reciprocal_approx_fast(rp[p][:], dn[p][:])
            nc.vector.tensor_tensor(
                ynrm[p][:].rearrange("q (b f) -> q b f", f=S)[:, 0:7],
                yb[:, 0:7 * SA].rearrange("q (b f) -> q b f", f=SA)[:, :, 0:S],
                rp[p][:, 0:7, None].broadcast_to((128, 7, S)),
                op=Alu.mult)
            nc.vector.tensor_tensor(
                ynrm[p][:, 7 * S:8 * S],
                yb[:, 512:512 + S],
                rp[p][:, 7:8].broadcast_to((128, S)),
                op=Alu.mult)
            for b in range(NB):
                nc.tensor.transpose(
                    ytp[:, 128 * b:128 * (b + 1)],
                    ynrm[p][:, S * b:S * (b + 1)],
                    identb, tile_position=(0, 0))
            nc.vector.tensor_copy(ynT[p][:], ytp[:])
            for i in range(2):
                nc.tensor.matmul(hT[:, 512 * i:512 * (i + 1)], Ws[p],
                                 ynT[p][:, 512 * i:512 * (i + 1)],
                                 start=True, stop=True)
            nc.scalar.activation(sg[p][:], hT[:], Act.Sigmoid,
                                 bias=biases[p])

        front("u", 0)
        front("d", 1)
        # prefetch the sigmoid table set while PE aggregates (ACT idle)
        nc.scalar.activation(dum[:], cf[0:1, 0:1], Act.Sigmoid)
        agg("u")
        norm_proj("u")
        agg("d")
        norm_proj("d")

        nc.vector.tensor_tensor(prod[:], sg["u"][:], sg["d"][:], op=Alu.mult)
        for i in range(2):
            nc.tensor.matmul(hT[0:1, 512 * i:512 * (i + 1)], mlpW,
                             prod[:, 512 * i:512 * (i + 1)],
                             start=True, stop=True)
            nc.vector.tensor_scalar(
                o_sb[:, 512 * i:512 * (i + 1)],
                hT[0:1, 512 * i:512 * (i + 1)], mlpb, None, op0=Alu.add)
        nc.sync.dma_start(out_dram.ap(), o_sb[:])

    nc.compile()
    return nc


def _get_module():
    if "nc" not in _CACHE:
        _CACHE["nc"] = _build_module()
    return _CACHE["nc"]


def _nz_counts(ei):
    src = np.asarray(ei[0]).astype(np.int64)
    dst = np.asarray(ei[1]).astype(np.int64)
    valid = (dst % NPG) == (NPG - 1)
    return np.bincount(src[valid], minlength=N).astype(np.float32)


def _pack_core(cnt_u, cnt_d, base):
    """First-fit-decreasing whole-graph packing for one core (shared
    graph->tile map for both branches)."""
    ku = [np.nonzero(cnt_u[base + NPG * g: base + NPG * (g + 1)])[0]
          for g in range(GPC)]
    kd = [np.nonzero(cnt_d[base + NPG * g: base + NPG * (g + 1)])[0]
          for g in range(GPC)]
    nu = np.array([len(k) for k in ku])
    nd = np.array([len(k) for k in kd])
    order = np.argsort(-(nu + nd), kind="stable")
    bins = []
    for g in order:
        placed = False
        for b in bins:
            if b[0] + nu[g] <= 128 and b[1] + nd[g] <= 128 and len(b[2]) < 32:
                b[0] += nu[g]
                b[1] += nd[g]
                b[2].append(g)
                placed = True
                break
        if not placed:
            bins.append([nu[g], nd[g], [g]])
    assert len(bins) <= T, f"packing needs {len(bins)} tiles > {T}"
    return [b[2] for b in bins], ku, kd


def _branch_arrays(tiles, klists, cnt, x, base):
    import ml_dtypes
    bf = ml_dtypes.bfloat16
    f8 = ml_dtypes.float8_e4m3
    SLOTS = T * 128
    xs = np.zeros((SLOTS, S), np.float32)
    cs = np.zeros(SLOTS, np.float32)
    mk = np.zeros((T, 128, 32), np.float32)
    mkT = np.zeros((128, NB * 128), np.float32)
    for t, gs in enumerate(tiles):
        off = 0
        for j, g in enumerate(gs):
            nodes = base + NPG * g + klists[g]
            k = len(nodes)
            xs[128 * t + off:128 * t + off + k] = x[nodes]
            cs[128 * t + off:128 * t + off + k] = cnt[nodes]
            mk[t, off:off + k, j] = 1.0
            mkT[32 * (t % 4) + j, 128 * (t // 4) + off:
                128 * (t // 4) + off + k] = 1.0
            off += k

    xab = np.zeros((T, 128, SA), np.float32)
    xab[:, :, :S] = xs.reshape(T, 128, S)
    xab[:, :, S] = 1.0
    xab = np.ascontiguousarray(
        xab.transpose(1, 0, 2).reshape(128, T * SA)).astype(bf)
    # xt: 2-block chunks — chunk c = even tile 2c (rows 0:64) and odd
    # tile 2c+1 (rows 64:128), feat-major
    xst = xs.reshape(T, 128, S)
    xtv = np.concatenate([
        xst[0::2].reshape(NCH * 128, S).T,
        xst[1::2].reshape(NCH * 128, S).T], axis=0)
    xtv = np.ascontiguousarray(xtv).astype(bf)
    cnt_t = np.ascontiguousarray(cs.reshape(T, 128).T)
    mk2 = np.ascontiguousarray(
        mk.transpose(1, 0, 2).reshape(128, T * 32)).astype(bf)
    return {"xab": xab, "xt": xtv, "mk": mk2, "mkT": mkT.astype(bf),
            "cnt": cnt_t}


def _build_in_maps(inputs):
    import ml_dtypes
    bf = ml_dtypes.bfloat16
    data = {}
    for p, pref in (("u", "up"), ("d", "down")):
        W = np.asarray(inputs[f"{pref}_W"], np.float32)
        data[p] = {
            "x": np.asarray(inputs[f"{pref}_x"], np.float32),
            "cnt": _nz_counts(inputs[f"{pref}_edge_index"]),
            "ws": W @ np.asarray(inputs[f"{pref}_att_src"], np.float32),
            "wd": W @ np.asarray(inputs[f"{pref}_att_dst"], np.float32),
            "W": W,
            "bias": np.asarray(inputs[f"{pref}_bias"], np.float32).reshape(H),
        }

    cb = np.zeros((128, CBW), np.float32)
    cb[:, CB_ID:CB_ID + 128] = np.eye(128)
    cb[0:S, CB_WSU:CB_WSU + 128] = data["u"]["W"]
    cb[0:S, CB_WSD:CB_WSD + 128] = data["d"]["W"]
    cb[0:S, CB_V2U] = data["u"]["ws"]
    cb[S:, CB_V2U + 1] = data["u"]["ws"]
    cb[0:S, CB_V2D] = data["d"]["ws"]
    cb[S:, CB_V2D + 1] = data["d"]["ws"]
    cb[0:S, CB_D2U] = data["u"]["wd"]
    cb[0:S, CB_D2D] = data["d"]["wd"]
    cb[:, CB_MLP] = np.asarray(inputs["mlp_W"], np.float32).reshape(H)
    cb[np.arange(128), CB_BM4 + np.arange(128) // 32] = 1.0
    cb = cb.astype(bf)

    cf = np.zeros((128, CFW), np.float32)
    cf[:, CF_EPS] = 1e-16
    cf[0, CF_MLB] = float(np.asarray(inputs["mlp_b"]).reshape(-1)[0])
    cf[:, CF_BU] = data["u"]["bias"]
    cf[:, CF_BD] = data["d"]["bias"]

    in_maps = []
    perms = []
    for c in range(NC):
        base = c * NPC
        tiles, ku, kd = _pack_core(data["u"]["cnt"], data["d"]["cnt"], base)
        gd_of_g = np.full(GPC, -1, np.int64)
        for t, gs in enumerate(tiles):
            for j, g in enumerate(gs):
                gd_of_g[g] = 32 * t + j
        assert (gd_of_g >= 0).all()
        perms.append(gd_of_g)
        m = {"cb": cb}
        cfc = cf.copy()
        for p, kl in (("u", ku), ("d", kd)):
            arrs = _branch_arrays(tiles, kl, data[p]["cnt"],
                                  data[p]["x"], base)
            for kk in ("xab", "xt", "mk", "mkT"):
                m[f"{p}_{kk}"] = arrs[kk]
            off = CF_CNT if p == "u" else CF_CNT + T
            cfc[:, off:off + T] = arrs["cnt"]
            # x_last feat-major by gd slot
            xlT = np.zeros((S, GD), np.float32)
            gl = np.arange(GPC)
            last = data[p]["x"][base + NPG * gl + NPG - 1]   # [GPC, S]
            xlT[:, gd_of_g] = last.T
            m[f"{p}_xlT"] = xlT.astype(bf)
        m["cf"] = cfc
        in_maps.append(m)
    return in_maps, perms


def kernel(**inputs):
    from concourse.bass_utils import run_bass_kernel_spmd

    nc = _get_module()
    in_maps, perms = _build_in_maps(inputs)
    res = run_bass_kernel_spmd(nc, in_maps, core_ids=list(range(NC)))
    out = np.empty((NC, GPC), np.float32)
    for c, r in enumerate(res.results):
        full = np.asarray(r["out"], np.float32).reshape(GD)
        out[c] = full[perms[c]]
    return out.reshape(G, 1)


# revision 11
# speedup vs baseline: 1.3026x; 1.0847x over previous
"""Trainium2 Bass kernel for nn_Event_Critic_Net (dual-branch GAT critic).

Math: the reference reads the GAT output only at the LAST node of each
graph (graphs are 32 contiguous nodes), so only edges whose dst is a
graph's last node contribute:

    out_g = sigmoid( (sum_n alpha[n] * x[n,:]) @ W + bias )
    alpha[n] = cnt[n]*exp(e[n]) / (sum_n cnt[n]*exp(e[n]) + 1e-16)
    e[n] = leaky_relu(x[n]. w_src + x[last(g)]. w_dst),  w_* = W @ att_*

cnt[n] = #edges (n -> last(g)).  Nodes with cnt==0 are dropped on the
host; survivors (~7/graph) are packed whole-graph into 128-slot tiles
via first-fit-decreasing (T=32 tiles/core/branch, measured need <=30).

Device pipeline per branch (u then d, pipelined across engines):
  1. advb MMs: a_dst per gd slot (xlT chunks x wd2) -> PSUM.
  2. a_src MMs: 16 two-block xt chunks x wv2 -> e PSUM (start, no stop)
  3. advs copy (ACT), advm = advs*Bm4 (DVE), 8 mkT scatter MMs
     accumulate a_dst INTO the same e PSUM cols (stop=True).
  4. leaky (DVE from PSUM), exp (ACT), P = cnt*exp (DVE).
  5. mkP = mk (*) P : mask columns scaled by P (DVE); aggregation MMs
     use mkP stationary x raw xab tiles -> yps (ones col -> denom).
  6. normalize (DVE from PSUM), 8 PE transposes -> ynT, project (2 MMs),
     sigmoid (ACT; table-set load prefetched right after the last exp
     via a dummy 1-elem sigmoid), prod (DVE), mlp (2 MMs), bias, DMA.

DMA: 3 queues (sync/scalar HWDGE, gpsimd SWDGE); masks shipped as
fp8_e4m3 and cast to bf16 during the gpsimd DMA (halves their bytes).
Host un-permutes via the packing's graph->gd map (gd = out column).
"""

import numpy as np
from contextlib import ExitStack

NC = 8            # cores
N = 131072        # nodes total
G = 4096          # graphs
NPG = 32          # nodes per graph
S = 64            # state size
H = 128           # hidden size
NPC = N // NC     # 16384 nodes per core
GPC = G // NC     # 512 graphs per core
SA = 66           # x columns: 64 features | ones | zero pad
T = 32            # packed node tiles per core per branch (need <=30)
NB = T // 4       # 8 mkT blocks / gd blocks of 128
NCH = T // 2      # 16 two-block xt chunks
GD = NB * 128     # 1024 gd slots
NEG = 0.2

# cb (bf16) column layout
CB_ID = 0                      # identity [128,128]
CB_WSU = CB_ID + 128           # Ws_u rows 0:64 [*,128]
CB_WSD = CB_WSU + 128
CB_V2U = CB_WSD + 128          # wv2_u [128,2] (2-block ws)
CB_V2D = CB_V2U + 2
CB_D2U = CB_V2D + 2            # wd2_u [64,1]
CB_D2D = CB_D2U + 1
CB_MLP = CB_D2D + 1            # mlpW [128,1]
CB_BM4 = CB_MLP + 1            # Bm4 [128,4]
CBW = CB_BM4 + 4

# cf (f32) column layout
CF_EPS = 0                     # 1e-16
CF_MLB = 1                     # mlp_b
CF_BU = 2                      # bias_u per-partition
CF_BD = 3
CF_CNT = 4                     # cnt_u [128,T] then cnt_d [128,T]
CFW = CF_CNT + 2 * T

_CACHE = {}


def _build_module():
    import concourse.tile as tile
    from concourse import bacc, mybir
    from concourse.alu_op_type import AluOpType as Alu

    f32 = mybir.dt.float32
    bf16 = mybir.dt.bfloat16
    f8 = mybir.dt.float8e4
    Act = mybir.ActivationFunctionType

    nc = bacc.Bacc("TRN2", target_bir_lowering=False, debug=False,
                   num_devices=NC)

    dram = {}

    def din(name, shape, dt=bf16):
        dram[name] = nc.dram_tensor(name, shape, dt, kind="ExternalInput")

    for p in ("u", "d"):
        din(f"{p}_xab", [128, T * SA])
        din(f"{p}_xt", [128, NCH * 128])
        din(f"{p}_xlT", [64, GD])
        din(f"{p}_mk", [128, T * 32], f8)
        din(f"{p}_mkT", [128, NB * 128], f8)
    din("cb", [128, CBW])
    din("cf", [128, CFW], f32)
    out_dram = nc.dram_tensor("out", [1, GD], f32, kind="ExternalOutput")

    with tile.TileContext(nc) as tc, ExitStack() as ctx:
        const = ctx.enter_context(tc.tile_pool(name="const", bufs=1))
        xp = ctx.enter_context(tc.tile_pool(name="xp", bufs=1))
        wk = ctx.enter_context(tc.tile_pool(name="wk", bufs=1))
        ps1 = ctx.enter_context(tc.tile_pool(name="ps1", bufs=1, space="PSUM"))

        cb = const.tile([128, CBW], bf16, tag="cb")
        cf = const.tile([128, CFW], f32, tag="cf")
        st = {"u": {}, "d": {}}
        for p in ("u", "d"):
            s = st[p]
            s["xab"] = xp.tile([128, T * SA], bf16, tag=f"xab_{p}",
                               name=f"xab_{p}")
            s["xt"] = xp.tile([128, NCH * 128], bf16, tag=f"xt_{p}",
                              name=f"xt_{p}")
            s["xlT"] = xp.tile([64, GD], bf16, tag=f"xlT_{p}",
                               name=f"xlT_{p}")
            s["mk"] = xp.tile([128, T * 32], bf16, tag=f"mk_{p}",
                              name=f"mk_{p}")
            s["mkT"] = xp.tile([128, NB * 128], bf16, tag=f"mkT_{p}",
                               name=f"mkT_{p}")

        # ---- DMA enqueues (3 queues; front tensors first) ----
        nc.sync.dma_start(cf[:], dram["cf"].ap())
        nc.sync.dma_start(cb[:], dram["cb"].ap())
        nc.sync.dma_start(st["u"]["xlT"][:], dram["u_xlT"].ap())
        nc.sync.dma_start(st["u"]["xt"][:], dram["u_xt"].ap())
        nc.sync.dma_start(st["d"]["xlT"][:], dram["d_xlT"].ap())
        nc.scalar.dma_start(st["u"]["xab"][:], dram["u_xab"].ap())
        nc.scalar.dma_start(st["d"]["xt"][:], dram["d_xt"].ap())
        nc.gpsimd.dma_start(st["u"]["mkT"][:], dram["u_mkT"].ap())
        nc.gpsimd.dma_start(st["u"]["mk"][:], dram["u_mk"].ap())
        nc.gpsimd.dma_start(st["d"]["mkT"][:], dram["d_mkT"].ap())
        nc.gpsimd.dma_start(st["d"]["mk"][:], dram["d_mk"].ap())
        nc.gpsimd.dma_start(st["d"]["xab"][:], dram["d_xab"].ap())

        identb = cb[:, CB_ID:CB_ID + 128]
        Ws = {"u": cb[0:S, CB_WSU:CB_WSU + 128],
              "d": cb[0:S, CB_WSD:CB_WSD + 128]}
        wv2 = {"u": cb[:, CB_V2U:CB_V2U + 2], "d": cb[:, CB_V2D:CB_V2D + 2]}
        wd2 = {"u": cb[0:S, CB_D2U:CB_D2U + 1],
               "d": cb[0:S, CB_D2D:CB_D2D + 1]}
        mlpW = cb[:, CB_MLP:CB_MLP + 1]
        Bm4 = cb[:, CB_BM4:CB_BM4 + 4]
        eps = cf[:, CF_EPS:CF_EPS + 1]
        mlpb = cf[0:1, CF_MLB:CF_MLB + 1]
        biases = {"u": cf[:, CF_BU:CF_BU + 1], "d": cf[:, CF_BD:CF_BD + 1]}
        cnts = {"u": cf[:, CF_CNT:CF_CNT + T],
                "d": cf[:, CF_CNT + T:CF_CNT + 2 * T]}

        # ---- PSUM tiles (8 banks exactly) ----
        # epsud: e_u | e_d | advb_u | advb_d
        epsud = ps1.tile([128, 4 * T + 2 * NB], f32, tag="epsud")
        # per-branch y PSUM: blocks 0..6 at col 66*b, block 7 at col 512
        yps = {"u": ps1.tile([128, 578], f32, tag="yps_u", name="yps_u"),
               "d": ps1.tile([128, 578], f32, tag="yps_d", name="yps_d")}
        ytp = ps1.tile([64, NB * 128], bf16, tag="ytp")      # shared u->d
        hT = ps1.tile([128, 1024], f32, tag="hT")            # proj + mlp

        def ycol(b):
            return SA * b if b < 7 else 512

        # ---- SBUF work tiles ----
        advs = {p: wk.tile([128, NB], f32, tag=f"advs_{p}", name=f"advs_{p}")
                for p in "ud"}
        advm = {p: wk.tile([128, NB * 4], bf16, tag=f"advm_{p}",
                           name=f"advm_{p}") for p in "ud"}
        ee = {p: wk.tile([128, T], f32, tag=f"ee_{p}", name=f"ee_{p}")
              for p in "ud"}
        es = {p: wk.tile([128, T], f32, tag=f"es_{p}", name=f"es_{p}")
              for p in "ud"}
        adsb = {p: wk.tile([128, T], f32, tag=f"adsb_{p}", name=f"adsb_{p}")
                for p in "ud"}
        ex = {p: wk.tile([128, T], f32, tag=f"ex_{p}", name=f"ex_{p}")
              for p in "ud"}
        Pt = {p: wk.tile([128, T], f32, tag=f"P_{p}", name=f"P_{p}")
              for p in "ud"}
        mkP = {p: wk.tile([128, T * 32], bf16, tag=f"mkP_{p}",
                          name=f"mkP_{p}") for p in "ud"}
        dn = {p: wk.tile([128, NB], f32, tag=f"dn_{p}", name=f"dn_{p}")
              for p in "ud"}
        rp = {p: wk.tile([128, NB], f32, tag=f"rp_{p}", name=f"rp_{p}")
              for p in "ud"}
        ynrm = {p: wk.tile([128, NB * S], bf16, tag=f"ynrm_{p}",
                           name=f"ynrm_{p}") for p in "ud"}
        ynT = {p: wk.tile([64, GD], bf16, tag=f"ynT_{p}", name=f"ynT_{p}")
               for p in "ud"}
        sg = {p: wk.tile([128, GD], bf16, tag=f"sg_{p}", name=f"sg_{p}")
              for p in "ud"}
        prod = wk.tile([128, GD], bf16, tag="prod")
        o_sb = wk.tile([1, GD], f32, tag="o_sb")

        def front(p, po):
            s = st[p]
            # a_dst per gd: 8 xlT-chunk MMs -> advb cols
            for b in range(NB):
                nc.tensor.matmul(
                    epsud[:, 4 * T + NB * po + b:4 * T + NB * po + b + 1],
                    s["xlT"][:, 128 * b:128 * (b + 1)],
                    wd2[p], start=True, stop=True)
            # a_src: 16 chunk MMs open the e accumulation group
            for c in range(NCH):
                nc.tensor.matmul(
                    epsud[:, T * po + 2 * c:T * po + 2 * c + 2],
                    s["xt"][:, 128 * c:128 * (c + 1)],
                    wv2[p], start=True, stop=True)
            # advs (ACT copy from PSUM), advm = advs * Bm4 (DVE)
            nc.vector.tensor_scalar(
                advs[p][:], epsud[:, 4 * T + NB * po:4 * T + NB * (po + 1)],
                0.0, None, op0=Alu.add)
            nc.vector.tensor_tensor(
                advm[p][:].rearrange("q (b n) -> q b n", n=4),
                advs[p][:, :, None].broadcast_to((128, NB, 4)),
                Bm4[:, None, :].broadcast_to((128, NB, 4)),
                op=Alu.mult)
            # scatter a_dst into the e cols (accumulate, close group)
            for b in range(NB):
                nc.tensor.matmul(
                    epsud[:, 2 * T + T * po + 4 * b:2 * T + T * po + 4 * b + 4],
                    s["mkT"][:, 128 * b:128 * (b + 1)],
                    advm[p][:, 4 * b:4 * b + 4],
                    start=True, stop=True)
            # e = e_src + e_dst; P = cnt * exp(leaky(e))
            nc.vector.tensor_scalar(
                adsb[p][:], epsud[:, 2 * T + T * po:2 * T + T * (po + 1)],
                0.0, None, op0=Alu.add)
            nc.vector.scalar_tensor_tensor(
                es[p][:], epsud[:, T * po:T * (po + 1)], 1.0, adsb[p][:],
                op0=Alu.mult, op1=Alu.add)
            nc.vector.scalar_tensor_tensor(
                ee[p][:], es[p][:], NEG, es[p][:], op0=Alu.mult, op1=Alu.max)
            nc.scalar.activation(ex[p][:], ee[p][:], Act.Exp)
            nc.vector.tensor_tensor(Pt[p][:], ex[p][:], cnts[p], op=Alu.mult)
            # mkP = mk (*) P  (mask columns scaled by P)
            nc.vector.tensor_tensor(
                mkP[p][:].rearrange("q (t j) -> q t j", j=32),
                s["mk"][:].rearrange("q (t j) -> q t j", j=32),
                Pt[p][:, :, None].broadcast_to((128, T, 32)),
                op=Alu.mult)

        def agg(p):
            s = st[p]
            yb = yps[p]
            for t in range(T):
                nc.tensor.matmul(
                    yb[32 * (t % 4):32 * (t % 4) + 32,
                       ycol(t // 4):ycol(t // 4) + SA],
                    mkP[p][:, 32 * t:32 * (t + 1)],
                    s["xab"][:, SA * t:SA * (t + 1)],
                    start=True, stop=True,
                    tile_position=(0, 32 * (t % 4)))

        def norm_proj(p):
            yb = yps[p]
            nc.vector.tensor_scalar(
                dn[p][:, 0:7],
                yb[:, 0:7 * SA].rearrange("q (b f) -> q b f", f=SA)[:, :, S],
                eps, None, op0=Alu.add)
            nc.vector.tensor_scalar(
                dn[p][:, 7:8], yb[:, 512 + S:512 + S + 1], eps, None,
                op0=Alu.add)
            nc.vector.reciprocal_approx_fast(rp[p][:], dn[p][:])
            nc.vector.tensor_tensor(
                ynrm[p][:].rearrange("q (b f) -> q b f", f=S)[:, 0:7],
                yb[:, 0:7 * SA].rearrange("q (b f) -> q b f", f=SA)[:, :, 0:S],
                rp[p][:, 0:7, None].broadcast_to((128, 7, S)),
                op=Alu.mult)
            nc.vector.tensor_tensor(
                ynrm[p][:, 7 * S:8 * S],
                yb[:, 512:512 + S],
                rp[p][:, 7:8].broadcast_to((128, S)),
                op=Alu.mult)
            for b in range(NB):
                nc.tensor.transpose(
                    ytp[:, 128 * b:128 * (b + 1)],
                    ynrm[p][:, S * b:S * (b + 1)],
                    identb, tile_position=(0, 0))
            nc.vector.tensor_copy(ynT[p][:], ytp[:])
            for i in range(2):
                nc.tensor.matmul(hT[:, 512 * i:512 * (i + 1)], Ws[p],
                                 ynT[p][:, 512 * i:512 * (i + 1)],
                                 start=True, stop=True)
            nc.scalar.activation(sg[p][:], hT[:], Act.Tanh,
                                 bias=biases[p], scale=0.5)

        front("u", 0)
        front("d", 1)
        agg("u")
        norm_proj("u")
        agg("d")
        norm_proj("d")

        nc.vector.tensor_tensor(prod[:], sg["u"][:], sg["d"][:], op=Alu.mult)
        for i in range(2):
            for k, rhs in enumerate((prod, sg["u"], sg["d"])):
                nc.tensor.matmul(hT[0:1, 512 * i:512 * (i + 1)], mlpW,
                                 rhs[:, 512 * i:512 * (i + 1)],
                                 start=(k == 0), stop=(k == 2))
            nc.vector.tensor_scalar(
                o_sb[:, 512 * i:512 * (i + 1)],
                hT[0:1, 512 * i:512 * (i + 1)], mlpb, None, op0=Alu.add)
        nc.sync.dma_start(out_dram.ap(), o_sb[:])

    nc.compile()
    return nc


def _get_module():
    if "nc" not in _CACHE:
        _CACHE["nc"] = _build_module()
    return _CACHE["nc"]


def _nz_counts(ei):
    src = np.asarray(ei[0]).astype(np.int64)
    dst = np.asarray(ei[1]).astype(np.int64)
    valid = (dst % NPG) == (NPG - 1)
    return np.bincount(src[valid], minlength=N).astype(np.float32)


def _pack_core(cnt_u, cnt_d, base):
    """First-fit-decreasing whole-graph packing for one core (shared
    graph->tile map for both branches)."""
    ku = [np.nonzero(cnt_u[base + NPG * g: base + NPG * (g + 1)])[0]
          for g in range(GPC)]
    kd = [np.nonzero(cnt_d[base + NPG * g: base + NPG * (g + 1)])[0]
          for g in range(GPC)]
    nu = np.array([len(k) for k in ku])
    nd = np.array([len(k) for k in kd])
    order = np.argsort(-(nu + nd), kind="stable")
    bins = []
    for g in order:
        placed = False
        for b in bins:
            if b[0] + nu[g] <= 128 and b[1] + nd[g] <= 128 and len(b[2]) < 32:
                b[0] += nu[g]
                b[1] += nd[g]
                b[2].append(g)
                placed = True
                break
        if not placed:
            bins.append([nu[g], nd[g], [g]])
    assert len(bins) <= T, f"packing needs {len(bins)} tiles > {T}"
    return [b[2] for b in bins], ku, kd


def _branch_arrays(tiles, klists, cnt, x, base):
    import ml_dtypes
    bf = ml_dtypes.bfloat16
    f8 = ml_dtypes.float8_e4m3
    SLOTS = T * 128
    xs = np.zeros((SLOTS, S), np.float32)
    cs = np.zeros(SLOTS, np.float32)
    mk = np.zeros((T, 128, 32), np.float32)
    mkT = np.zeros((128, NB * 128), np.float32)
    for t, gs in enumerate(tiles):
        off = 0
        for j, g in enumerate(gs):
            nodes = base + NPG * g + klists[g]
            k = len(nodes)
            xs[128 * t + off:128 * t + off + k] = x[nodes]
            cs[128 * t + off:128 * t + off + k] = cnt[nodes]
            mk[t, off:off + k, j] = 1.0
            mkT[32 * (t % 4) + j, 128 * (t // 4) + off:
                128 * (t // 4) + off + k] = 1.0
            off += k

    xab = np.zeros((T, 128, SA), np.float32)
    xab[:, :, :S] = xs.reshape(T, 128, S)
    xab[:, :, S] = 1.0
    xab = np.ascontiguousarray(
        xab.transpose(1, 0, 2).reshape(128, T * SA)).astype(bf)
    # xt: 2-block chunks — chunk c = even tile 2c (rows 0:64) and odd
    # tile 2c+1 (rows 64:128), feat-major
    xst = xs.reshape(T, 128, S)
    xtv = np.concatenate([
        xst[0::2].reshape(NCH * 128, S).T,
        xst[1::2].reshape(NCH * 128, S).T], axis=0)
    xtv = np.ascontiguousarray(xtv).astype(bf)
    cnt_t = np.ascontiguousarray(cs.reshape(T, 128).T)
    mk2 = np.ascontiguousarray(
        mk.transpose(1, 0, 2).reshape(128, T * 32)).astype(f8)
    return {"xab": xab, "xt": xtv, "mk": mk2, "mkT": mkT.astype(f8),
            "cnt": cnt_t}


def _build_in_maps(inputs):
    import ml_dtypes
    bf = ml_dtypes.bfloat16
    data = {}
    for p, pref in (("u", "up"), ("d", "down")):
        W = np.asarray(inputs[f"{pref}_W"], np.float32)
        data[p] = {
            "x": np.asarray(inputs[f"{pref}_x"], np.float32),
            "cnt": _nz_counts(inputs[f"{pref}_edge_index"]),
            "ws": W @ np.asarray(inputs[f"{pref}_att_src"], np.float32),
            "wd": W @ np.asarray(inputs[f"{pref}_att_dst"], np.float32),
            "W": W,
            "bias": np.asarray(inputs[f"{pref}_bias"], np.float32).reshape(H),
        }

    cb = np.zeros((128, CBW), np.float32)
    cb[:, CB_ID:CB_ID + 128] = np.eye(128)
    cb[0:S, CB_WSU:CB_WSU + 128] = data["u"]["W"]
    cb[0:S, CB_WSD:CB_WSD + 128] = data["d"]["W"]
    cb[0:S, CB_V2U] = data["u"]["ws"]
    cb[S:, CB_V2U + 1] = data["u"]["ws"]
    cb[0:S, CB_V2D] = data["d"]["ws"]
    cb[S:, CB_V2D + 1] = data["d"]["ws"]
    cb[0:S, CB_D2U] = data["u"]["wd"]
    cb[0:S, CB_D2D] = data["d"]["wd"]
    mw = np.asarray(inputs["mlp_W"], np.float32).reshape(H)
    cb[:, CB_MLP] = 0.25 * mw
    cb[np.arange(128), CB_BM4 + np.arange(128) // 32] = 1.0
    cb = cb.astype(bf)

    cf = np.zeros((128, CFW), np.float32)
    cf[:, CF_EPS] = 1e-16
    mw = np.asarray(inputs["mlp_W"], np.float32).reshape(H)
    cf[0, CF_MLB] = float(np.asarray(inputs["mlp_b"]).reshape(-1)[0]) \
        + 0.25 * float(mw.sum())
    cf[:, CF_BU] = 0.5 * data["u"]["bias"]
    cf[:, CF_BD] = 0.5 * data["d"]["bias"]

    in_maps = []
    perms = []
    for c in range(NC):
        base = c * NPC
        tiles, ku, kd = _pack_core(data["u"]["cnt"], data["d"]["cnt"], base)
        gd_of_g = np.full(GPC, -1, np.int64)
        for t, gs in enumerate(tiles):
            for j, g in enumerate(gs):
                gd_of_g[g] = 32 * t + j
        assert (gd_of_g >= 0).all()
        perms.append(gd_of_g)
        m = {"cb": cb}
        cfc = cf.copy()
        for p, kl in (("u", ku), ("d", kd)):
            arrs = _branch_arrays(tiles, kl, data[p]["cnt"],
                                  data[p]["x"], base)
            for kk in ("xab", "xt", "mk", "mkT"):
                m[f"{p}_{kk}"] = arrs[kk]
            off = CF_CNT if p == "u" else CF_CNT + T
            cfc[:, off:off + T] = arrs["cnt"]
            # x_last feat-major by gd slot
            xlT = np.zeros((S, GD), np.float32)
            gl = np.arange(GPC)
            last = data[p]["x"][base + NPG * gl + NPG - 1]   # [GPC, S]
            xlT[:, gd_of_g] = last.T
            m[f"{p}_xlT"] = xlT.astype(bf)
        m["cf"] = cfc
        in_maps.append(m)
    return in_maps, perms


def kernel(**inputs):
    from concourse.bass_utils import run_bass_kernel_spmd

    nc = _get_module()
    in_maps, perms = _build_in_maps(inputs)
    res = run_bass_kernel_spmd(nc, in_maps, core_ids=list(range(NC)))
    out = np.empty((NC, GPC), np.float32)
    for c, r in enumerate(res.results):
        full = np.asarray(r["out"], np.float32).reshape(GD)
        out[c] = full[perms[c]]
    return out.reshape(G, 1)


# revision 12
# speedup vs baseline: 1.3631x; 1.0464x over previous
"""Trainium2 Bass kernel for nn_Event_Critic_Net (dual-branch GAT critic).

Math: the reference reads the GAT output only at the LAST node of each
graph (graphs are 32 contiguous nodes), so only edges whose dst is a
graph's last node contribute:

    out_g = sigmoid( (sum_n alpha[n] * x[n,:]) @ W + bias )
    alpha[n] = cnt[n]*exp(e[n]) / (sum_n cnt[n]*exp(e[n]) + 1e-16)
    e[n] = leaky_relu(x[n]. w_src + x[last(g)]. w_dst),  w_* = W @ att_*

cnt[n] = #edges (n -> last(g)).  Nodes with cnt==0 are dropped on the
host; survivors (~7/graph) are packed whole-graph into 128-slot tiles
via first-fit-decreasing (T=32 tiles/core/branch, measured need <=30).

Device pipeline per branch (u then d, pipelined across engines):
  1. advb MMs: a_dst per gd slot (xlT chunks x wd2) -> PSUM.
  2. a_src MMs: 16 two-block xt chunks x wv2 -> e PSUM (start, no stop)
  3. advs copy (ACT), advm = advs*Bm4 (DVE), 8 mkT scatter MMs
     accumulate a_dst INTO the same e PSUM cols (stop=True).
  4. leaky (DVE from PSUM), exp (ACT), P = cnt*exp (DVE).
  5. mkP = mk (*) P : mask columns scaled by P (DVE); aggregation MMs
     use mkP stationary x raw xab tiles -> yps (ones col -> denom).
  6. normalize (DVE from PSUM), 8 PE transposes -> ynT, project (2 MMs),
     sigmoid (ACT; table-set load prefetched right after the last exp
     via a dummy 1-elem sigmoid), prod (DVE), mlp (2 MMs), bias, DMA.

DMA: 3 queues (sync/scalar HWDGE, gpsimd SWDGE); masks shipped as
fp8_e4m3 and cast to bf16 during the gpsimd DMA (halves their bytes).
Host un-permutes via the packing's graph->gd map (gd = out column).
"""

import numpy as np
from contextlib import ExitStack

NC = 8            # cores
N = 131072        # nodes total
G = 4096          # graphs
NPG = 32          # nodes per graph
S = 64            # state size
H = 128           # hidden size
NPC = N // NC     # 16384 nodes per core
GPC = G // NC     # 512 graphs per core
SA = 66           # x columns: 64 features | ones | zero pad
T = 32            # packed node tiles per core per branch (need <=30)
NB = T // 4       # 8 mkT blocks / gd blocks of 128
NCH = T // 2      # 16 two-block xt chunks
GD = NB * 128     # 1024 gd slots
NEG = 0.2

# cb (bf16) column layout
CB_ID = 0                      # identity [128,128]
CB_WSU = CB_ID + 128           # Ws_u rows 0:64 [*,128]
CB_WSD = CB_WSU + 128
CB_V2U = CB_WSD + 128          # wv2_u [128,2] (2-block ws)
CB_V2D = CB_V2U + 2
CB_D2U = CB_V2D + 2            # wd2_u [64,1]
CB_D2D = CB_D2U + 1
CB_MLP = CB_D2D + 1            # mlpW [128,1]
CB_BM4 = CB_MLP + 1            # Bm4 [128,4]
CBW = CB_BM4 + 4

# cf (f32) column layout
CF_EPS = 0                     # 1e-16
CF_MLB = 1                     # mlp_b
CF_BU = 2                      # bias_u per-partition
CF_BD = 3
CF_CNT = 4                     # cnt_u [128,T] then cnt_d [128,T]
CFW = CF_CNT + 2 * T

_CACHE = {}


def _build_module():
    import concourse.tile as tile
    from concourse import bacc, mybir
    from concourse.alu_op_type import AluOpType as Alu

    f32 = mybir.dt.float32
    bf16 = mybir.dt.bfloat16
    f8 = mybir.dt.float8e4
    Act = mybir.ActivationFunctionType

    nc = bacc.Bacc("TRN2", target_bir_lowering=False, debug=False,
                   num_devices=NC)

    dram = {}

    def din(name, shape, dt=bf16):
        dram[name] = nc.dram_tensor(name, shape, dt, kind="ExternalInput")

    for p in ("u", "d"):
        din(f"{p}_xab", [128, T * SA])
        din(f"{p}_xt", [128, NCH * 128])
        din(f"{p}_xlT", [64, GD])
        din(f"{p}_mk", [128, T * 32], f8)
        din(f"{p}_mkT", [128, NB * 128], f8)
    din("cb", [128, CBW])
    din("cf", [128, CFW], f32)
    out_dram = nc.dram_tensor("out", [1, GD], f32, kind="ExternalOutput")

    with tile.TileContext(nc) as tc, ExitStack() as ctx:
        const = ctx.enter_context(tc.tile_pool(name="const", bufs=1))
        xp = ctx.enter_context(tc.tile_pool(name="xp", bufs=1))
        wk = ctx.enter_context(tc.tile_pool(name="wk", bufs=1))
        ps1 = ctx.enter_context(tc.tile_pool(name="ps1", bufs=1, space="PSUM"))

        cb = const.tile([128, CBW], bf16, tag="cb")
        cf = const.tile([128, CFW], f32, tag="cf")
        st = {"u": {}, "d": {}}
        for p in ("u", "d"):
            s = st[p]
            s["xab"] = xp.tile([128, T * SA], bf16, tag=f"xab_{p}",
                               name=f"xab_{p}")
            s["xt"] = xp.tile([128, NCH * 128], bf16, tag=f"xt_{p}",
                              name=f"xt_{p}")
            s["xlT"] = xp.tile([64, GD], bf16, tag=f"xlT_{p}",
                               name=f"xlT_{p}")
            s["mk"] = xp.tile([128, T * 32], bf16, tag=f"mk_{p}",
                              name=f"mk_{p}")
            s["mkT"] = xp.tile([128, NB * 128], bf16, tag=f"mkT_{p}",
                               name=f"mkT_{p}")

        # ---- DMA enqueues (3 queues; front tensors first) ----
        nc.sync.dma_start(cf[:], dram["cf"].ap())
        nc.sync.dma_start(cb[:], dram["cb"].ap())
        nc.sync.dma_start(st["u"]["xlT"][:], dram["u_xlT"].ap())
        nc.sync.dma_start(st["u"]["xt"][:], dram["u_xt"].ap())
        nc.sync.dma_start(st["d"]["xlT"][:], dram["d_xlT"].ap())
        nc.scalar.dma_start(st["u"]["xab"][:], dram["u_xab"].ap())
        nc.scalar.dma_start(st["d"]["xt"][:], dram["d_xt"].ap())
        nc.gpsimd.dma_start(st["u"]["mkT"][:], dram["u_mkT"].ap())
        nc.gpsimd.dma_start(st["u"]["mk"][:], dram["u_mk"].ap())
        nc.gpsimd.dma_start(st["d"]["mkT"][:], dram["d_mkT"].ap())
        nc.gpsimd.dma_start(st["d"]["mk"][:], dram["d_mk"].ap())
        nc.gpsimd.dma_start(st["d"]["xab"][:], dram["d_xab"].ap())

        identb = cb[:, CB_ID:CB_ID + 128]
        Ws = {"u": cb[0:S, CB_WSU:CB_WSU + 128],
              "d": cb[0:S, CB_WSD:CB_WSD + 128]}
        wv2 = {"u": cb[:, CB_V2U:CB_V2U + 2], "d": cb[:, CB_V2D:CB_V2D + 2]}
        wd2 = {"u": cb[0:S, CB_D2U:CB_D2U + 1],
               "d": cb[0:S, CB_D2D:CB_D2D + 1]}
        mlpW = cb[:, CB_MLP:CB_MLP + 1]
        Bm4 = cb[:, CB_BM4:CB_BM4 + 4]
        eps = cf[:, CF_EPS:CF_EPS + 1]
        mlpb = cf[0:1, CF_MLB:CF_MLB + 1]
        biases = {"u": cf[:, CF_BU:CF_BU + 1], "d": cf[:, CF_BD:CF_BD + 1]}
        cnts = {"u": cf[:, CF_CNT:CF_CNT + T],
                "d": cf[:, CF_CNT + T:CF_CNT + 2 * T]}

        # ---- PSUM tiles (8 banks exactly) ----
        # epsud: e_u | e_d | advb_u | advb_d
        epsud = ps1.tile([128, 4 * T], f32, tag="epsud")
        # per-branch y PSUM: blocks 0..6 at col 66*b, block 7 at col 512
        yps = {"u": ps1.tile([128, 578 + NB], f32, tag="yps_u", name="yps_u"),
               "d": ps1.tile([128, 578 + NB], f32, tag="yps_d", name="yps_d")}
        ytp = ps1.tile([64, NB * 128], bf16, tag="ytp")      # shared u->d
        hT = ps1.tile([128, 1024], f32, tag="hT")            # proj + mlp

        def ycol(b):
            return SA * b if b < 7 else 512

        # ---- SBUF work tiles ----
        advm = {p: wk.tile([128, NB * 4], bf16, tag=f"advm_{p}",
                           name=f"advm_{p}") for p in "ud"}
        ee = {p: wk.tile([128, T], f32, tag=f"ee_{p}", name=f"ee_{p}")
              for p in "ud"}
        es = {p: wk.tile([128, T], f32, tag=f"es_{p}", name=f"es_{p}")
              for p in "ud"}
        adsb = {p: wk.tile([128, T], f32, tag=f"adsb_{p}", name=f"adsb_{p}")
                for p in "ud"}
        ex = {p: wk.tile([128, T], f32, tag=f"ex_{p}", name=f"ex_{p}")
              for p in "ud"}
        mkP = {p: wk.tile([128, T * 32], bf16, tag=f"mkP_{p}",
                          name=f"mkP_{p}") for p in "ud"}
        dn = {p: wk.tile([128, NB], f32, tag=f"dn_{p}", name=f"dn_{p}")
              for p in "ud"}
        rp = {p: wk.tile([128, NB], f32, tag=f"rp_{p}", name=f"rp_{p}")
              for p in "ud"}
        ynrm = {p: wk.tile([128, NB * S], bf16, tag=f"ynrm_{p}",
                           name=f"ynrm_{p}") for p in "ud"}
        ynT = {p: wk.tile([64, GD], bf16, tag=f"ynT_{p}", name=f"ynT_{p}")
               for p in "ud"}
        sg = {p: wk.tile([128, GD], bf16, tag=f"sg_{p}", name=f"sg_{p}")
              for p in "ud"}
        prod = wk.tile([128, GD], bf16, tag="prod")
        o_sb = wk.tile([1, GD], f32, tag="o_sb")

        def front(p, po):
            s = st[p]
            # a_dst per gd: 8 xlT-chunk MMs -> advb cols
            for b in range(NB):
                nc.tensor.matmul(
                    yps[p][:, 578 + b:578 + b + 1],
                    s["xlT"][:, 128 * b:128 * (b + 1)],
                    wd2[p], start=True, stop=True)
            # a_src: 16 chunk MMs open the e accumulation group
            for c in range(NCH):
                nc.tensor.matmul(
                    epsud[:, T * po + 2 * c:T * po + 2 * c + 2],
                    s["xt"][:, 128 * c:128 * (c + 1)],
                    wv2[p], start=True, stop=True)
            # advm[:, b, n] = advb[:, b] * Bm4[:, n]  (one fused stt)
            nc.vector.scalar_tensor_tensor(
                advm[p][:].rearrange("q (b n) -> q b n", n=4),
                yps[p][:, 578:578 + NB][:, :, None].broadcast_to(
                    (128, NB, 4)),
                1.0,
                Bm4[:, None, :].broadcast_to((128, NB, 4)),
                op0=Alu.mult, op1=Alu.mult)
            # scatter a_dst into the e cols (accumulate, close group)
            for b in range(NB):
                nc.tensor.matmul(
                    epsud[:, 2 * T + T * po + 4 * b:2 * T + T * po + 4 * b + 4],
                    s["mkT"][:, 128 * b:128 * (b + 1)],
                    advm[p][:, 4 * b:4 * b + 4],
                    start=True, stop=True)
            # e = e_src + e_dst; P = cnt * exp(leaky(e))
            nc.vector.tensor_scalar(
                adsb[p][:], epsud[:, 2 * T + T * po:2 * T + T * (po + 1)],
                0.0, None, op0=Alu.add)
            nc.vector.scalar_tensor_tensor(
                es[p][:], epsud[:, T * po:T * (po + 1)], 1.0, adsb[p][:],
                op0=Alu.mult, op1=Alu.add)
            nc.vector.scalar_tensor_tensor(
                ee[p][:], es[p][:], NEG, es[p][:], op0=Alu.mult, op1=Alu.max)
            nc.scalar.activation(ex[p][:], ee[p][:], Act.Exp)
            # mkP = (mk*cnt) (*) exp(e)   (cnt pre-folded on host)
            nc.vector.tensor_tensor(
                mkP[p][:].rearrange("q (t j) -> q t j", j=32),
                s["mk"][:].rearrange("q (t j) -> q t j", j=32),
                ex[p][:, :, None].broadcast_to((128, T, 32)),
                op=Alu.mult)

        def agg(p):
            s = st[p]
            yb = yps[p]
            for t in range(T):
                nc.tensor.matmul(
                    yb[32 * (t % 4):32 * (t % 4) + 32,
                       ycol(t // 4):ycol(t // 4) + SA],
                    mkP[p][:, 32 * t:32 * (t + 1)],
                    s["xab"][:, SA * t:SA * (t + 1)],
                    start=True, stop=True,
                    tile_position=(0, 32 * (t % 4)))

        def norm_proj(p):
            yb = yps[p]
            nc.vector.tensor_scalar(
                dn[p][:, 0:7],
                yb[:, 0:7 * SA].rearrange("q (b f) -> q b f", f=SA)[:, :, S],
                eps, None, op0=Alu.add)
            nc.vector.tensor_scalar(
                dn[p][:, 7:8], yb[:, 512 + S:512 + S + 1], eps, None,
                op0=Alu.add)
            nc.vector.reciprocal_approx_fast(rp[p][:], dn[p][:])
            nc.vector.tensor_tensor(
                ynrm[p][:].rearrange("q (b f) -> q b f", f=S)[:, 0:7],
                yb[:, 0:7 * SA].rearrange("q (b f) -> q b f", f=SA)[:, :, 0:S],
                rp[p][:, 0:7, None].broadcast_to((128, 7, S)),
                op=Alu.mult)
            nc.vector.tensor_tensor(
                ynrm[p][:, 7 * S:8 * S],
                yb[:, 512:512 + S],
                rp[p][:, 7:8].broadcast_to((128, S)),
                op=Alu.mult)
            for b in range(NB):
                nc.tensor.transpose(
                    ytp[:, 128 * b:128 * (b + 1)],
                    ynrm[p][:, S * b:S * (b + 1)],
                    identb, tile_position=(0, 0))
            nc.vector.tensor_copy(ynT[p][:], ytp[:])
            for i in range(2):
                nc.tensor.matmul(hT[:, 512 * i:512 * (i + 1)], Ws[p],
                                 ynT[p][:, 512 * i:512 * (i + 1)],
                                 start=True, stop=True)
            nc.scalar.activation(sg[p][:], hT[:], Act.Tanh,
                                 bias=biases[p], scale=0.5)

        front("u", 0)
        front("d", 1)
        agg("u")
        norm_proj("u")
        agg("d")
        norm_proj("d")

        nc.vector.tensor_tensor(prod[:], sg["u"][:], sg["d"][:], op=Alu.mult)
        for i in range(2):
            for k, rhs in enumerate((prod, sg["u"], sg["d"])):
                nc.tensor.matmul(hT[0:1, 512 * i:512 * (i + 1)], mlpW,
                                 rhs[:, 512 * i:512 * (i + 1)],
                                 start=(k == 0), stop=(k == 2))
            nc.vector.tensor_scalar(
                o_sb[:, 512 * i:512 * (i + 1)],
                hT[0:1, 512 * i:512 * (i + 1)], mlpb, None, op0=Alu.add)
        nc.sync.dma_start(out_dram.ap(), o_sb[:])

    nc.compile()
    return nc


def _get_module():
    if "nc" not in _CACHE:
        _CACHE["nc"] = _build_module()
    return _CACHE["nc"]


def _nz_counts(ei):
    src = np.asarray(ei[0]).astype(np.int64)
    dst = np.asarray(ei[1]).astype(np.int64)
    valid = (dst % NPG) == (NPG - 1)
    return np.bincount(src[valid], minlength=N).astype(np.float32)


def _pack_core(cnt_u, cnt_d, base):
    """First-fit-decreasing whole-graph packing for one core (shared
    graph->tile map for both branches)."""
    ku = [np.nonzero(cnt_u[base + NPG * g: base + NPG * (g + 1)])[0]
          for g in range(GPC)]
    kd = [np.nonzero(cnt_d[base + NPG * g: base + NPG * (g + 1)])[0]
          for g in range(GPC)]
    nu = np.array([len(k) for k in ku])
    nd = np.array([len(k) for k in kd])
    order = np.argsort(-(nu + nd), kind="stable")
    bins = []
    for g in order:
        placed = False
        for b in bins:
            if b[0] + nu[g] <= 128 and b[1] + nd[g] <= 128 and len(b[2]) < 32:
                b[0] += nu[g]
                b[1] += nd[g]
                b[2].append(g)
                placed = True
                break
        if not placed:
            bins.append([nu[g], nd[g], [g]])
    assert len(bins) <= T, f"packing needs {len(bins)} tiles > {T}"
    return [b[2] for b in bins], ku, kd


def _branch_arrays(tiles, klists, cnt, x, base):
    import ml_dtypes
    bf = ml_dtypes.bfloat16
    f8 = ml_dtypes.float8_e4m3
    SLOTS = T * 128
    xs = np.zeros((SLOTS, S), np.float32)
    cs = np.zeros(SLOTS, np.float32)
    mk = np.zeros((T, 128, 32), np.float32)
    mkT = np.zeros((128, NB * 128), np.float32)
    for t, gs in enumerate(tiles):
        off = 0
        for j, g in enumerate(gs):
            nodes = base + NPG * g + klists[g]
            k = len(nodes)
            xs[128 * t + off:128 * t + off + k] = x[nodes]
            cs[128 * t + off:128 * t + off + k] = cnt[nodes]
            mk[t, off:off + k, j] = cnt[nodes]
            mkT[32 * (t % 4) + j, 128 * (t // 4) + off:
                128 * (t // 4) + off + k] = 1.0
            off += k

    xab = np.zeros((T, 128, SA), np.float32)
    xab[:, :, :S] = xs.reshape(T, 128, S)
    xab[:, :, S] = 1.0
    xab = np.ascontiguousarray(
        xab.transpose(1, 0, 2).reshape(128, T * SA)).astype(bf)
    # xt: 2-block chunks — chunk c = even tile 2c (rows 0:64) and odd
    # tile 2c+1 (rows 64:128), feat-major
    xst = xs.reshape(T, 128, S)
    xtv = np.concatenate([
        xst[0::2].reshape(NCH * 128, S).T,
        xst[1::2].reshape(NCH * 128, S).T], axis=0)
    xtv = np.ascontiguousarray(xtv).astype(bf)
    cnt_t = np.ascontiguousarray(cs.reshape(T, 128).T)
    mk2 = np.ascontiguousarray(
        mk.transpose(1, 0, 2).reshape(128, T * 32)).astype(f8)
    return {"xab": xab, "xt": xtv, "mk": mk2, "mkT": mkT.astype(f8),
            "cnt": cnt_t}


def _build_in_maps(inputs):
    import ml_dtypes
    bf = ml_dtypes.bfloat16
    data = {}
    for p, pref in (("u", "up"), ("d", "down")):
        W = np.asarray(inputs[f"{pref}_W"], np.float32)
        data[p] = {
            "x": np.asarray(inputs[f"{pref}_x"], np.float32),
            "cnt": _nz_counts(inputs[f"{pref}_edge_index"]),
            "ws": W @ np.asarray(inputs[f"{pref}_att_src"], np.float32),
            "wd": W @ np.asarray(inputs[f"{pref}_att_dst"], np.float32),
            "W": W,
            "bias": np.asarray(inputs[f"{pref}_bias"], np.float32).reshape(H),
        }

    cb = np.zeros((128, CBW), np.float32)
    cb[:, CB_ID:CB_ID + 128] = np.eye(128)
    cb[0:S, CB_WSU:CB_WSU + 128] = data["u"]["W"]
    cb[0:S, CB_WSD:CB_WSD + 128] = data["d"]["W"]
    cb[0:S, CB_V2U] = data["u"]["ws"]
    cb[S:, CB_V2U + 1] = data["u"]["ws"]
    cb[0:S, CB_V2D] = data["d"]["ws"]
    cb[S:, CB_V2D + 1] = data["d"]["ws"]
    cb[0:S, CB_D2U] = data["u"]["wd"]
    cb[0:S, CB_D2D] = data["d"]["wd"]
    mw = np.asarray(inputs["mlp_W"], np.float32).reshape(H)
    cb[:, CB_MLP] = 0.25 * mw
    cb[np.arange(128), CB_BM4 + np.arange(128) // 32] = 1.0
    cb = cb.astype(bf)

    cf = np.zeros((128, CFW), np.float32)
    cf[:, CF_EPS] = 1e-16
    mw = np.asarray(inputs["mlp_W"], np.float32).reshape(H)
    cf[0, CF_MLB] = float(np.asarray(inputs["mlp_b"]).reshape(-1)[0]) \
        + 0.25 * float(mw.sum())
    cf[:, CF_BU] = 0.5 * data["u"]["bias"]
    cf[:, CF_BD] = 0.5 * data["d"]["bias"]

    in_maps = []
    perms = []
    for c in range(NC):
        base = c * NPC
        tiles, ku, kd = _pack_core(data["u"]["cnt"], data["d"]["cnt"], base)
        gd_of_g = np.full(GPC, -1, np.int64)
        for t, gs in enumerate(tiles):
            for j, g in enumerate(gs):
                gd_of_g[g] = 32 * t + j
        assert (gd_of_g >= 0).all()
        perms.append(gd_of_g)
        m = {"cb": cb}
        cfc = cf.copy()
        for p, kl in (("u", ku), ("d", kd)):
            arrs = _branch_arrays(tiles, kl, data[p]["cnt"],
                                  data[p]["x"], base)
            for kk in ("xab", "xt", "mk", "mkT"):
                m[f"{p}_{kk}"] = arrs[kk]
            off = CF_CNT if p == "u" else CF_CNT + T
            cfc[:, off:off + T] = arrs["cnt"]
            # x_last feat-major by gd slot
            xlT = np.zeros((S, GD), np.float32)
            gl = np.arange(GPC)
            last = data[p]["x"][base + NPG * gl + NPG - 1]   # [GPC, S]
            xlT[:, gd_of_g] = last.T
            m[f"{p}_xlT"] = xlT.astype(bf)
        m["cf"] = cfc
        in_maps.append(m)
    return in_maps, perms


def kernel(**inputs):
    from concourse.bass_utils import run_bass_kernel_spmd

    nc = _get_module()
    in_maps, perms = _build_in_maps(inputs)
    res = run_bass_kernel_spmd(nc, in_maps, core_ids=list(range(NC)))
    out = np.empty((NC, GPC), np.float32)
    for c, r in enumerate(res.results):
        full = np.asarray(r["out"], np.float32).reshape(GD)
        out[c] = full[perms[c]]
    return out.reshape(G, 1)
